# revision 9
# baseline (speedup 1.0000x reference)
"""Trainium2 Bass kernel for nn_AttentionMechanism (sparse_attention).

Reference computation (per full batch B=32):
    h          = hidden[0]                         # [B, H]
    proj       = h @ W.T                           # [B, D]
    scores     = einsum('cbd,bd->cb', ctx, proj)   # [C, B]
    scores     = where(mask, -inf, scores)
    attn       = softmax(scores, axis=0)           # [C, B]
    ctxout     = einsum('cb,cbd->bd', attn, ctx)   # [B, D]
    context    = broadcast ctxout  -> [T, B, D]
    attentions = broadcast attn.T  -> [T, B, C]

Sharding: data-parallel over batch across 8 NeuronCores (4 batches/core),
W replicated.  Per-core kernel keeps contextvects in natural layout
[c(part), d(free)]:
  - scores: fused multiply+reduce on DVE (contract d along free dim)
  - softmax: DVE row-max -> GPSIMD partition max -> ACT fused exp+rowsum
             -> PE ones-matmul partition sum is replaced by GPSIMD add
  - weighted sum: PE matmuls (contract c = partition dim), M=1, PSUM acc
  - seqlen broadcast done by step-0 source DMA from a DRAM bounce row
"""

import os
import sys

for _p in ("/opt/trn_rl_repo", "/root/.axon_site/_ro/trn_rl_repo"):
    if os.path.isdir(_p) and _p not in sys.path:
        sys.path.insert(0, _p)

import numpy as np

import concourse.bass as bass
import concourse.tile as tile
from concourse import mybir
import concourse.bass_isa as bass_isa
from concourse.bass_utils import run_bass_kernel_spmd
from concourse.masks import make_identity

F32 = mybir.dt.float32
U8 = mybir.dt.uint8

N_CORES = 8


_MAX_WAITS = 1


def _split_excess_waits(nc):
    """This container's walrus rejects instructions carrying more than ~2 sem
    waits.  Move excess waits onto same-engine NOPs inserted just before the
    offending instruction (waits still happen-before it in program order)."""
    n_split = 0
    for f in nc.m.functions:
        for bb in f.blocks:
            new_insts = []
            for ins in bb.instructions:
                si = ins.sync_info
                if si is not None and si.on_wait and len(si.on_wait) > _MAX_WAITS:
                    waits = list(si.on_wait)
                    keep = waits[: _MAX_WAITS]
                    rest = waits[_MAX_WAITS:]
                    si.on_wait.clear()
                    for w in keep:
                        si.on_wait.append(w)
                    for j, w in enumerate(rest):
                        nop = mybir.InstNoOp(
                            name=f"{ins.name}-wsplit{j}", ins=[], outs=[]
                        )
                        nop.engine = ins.engine
                        nop.sync_info = mybir.SyncInfo(on_wait=[w], on_update=[])
                        new_insts.append(nop)
                        n_split += 1
                new_insts.append(ins)
            bb.instructions.clear()
            for i in new_insts:
                bb.instructions.append(i)
    return n_split


def build_nc(T=64, C=2048, D=1024, H=1024, B=4, trace_sim=False):
    """Build the per-core bass program (B = batches on this core)."""
    assert C % 128 == 0 and D % 512 == 0 and H % 1024 == 0
    K = C // 128          # context tiles (partition dim c)
    HT = H // 128         # h contraction tiles
    DT = D // 128         # dout tiles of W (natural layout)
    NCH = D // 512        # 512-wide output chunks

    nc = bass.Bass()
    hid = nc.declare_dram_parameter("hidden", [1, B, H], F32, isOutput=False)
    cvec = nc.declare_dram_parameter("contextvects", [C, B, D], F32, isOutput=False)
    Wp = nc.declare_dram_parameter("W", [D, H], F32, isOutput=False)
    maskp = nc.declare_dram_parameter("padding_mask", [C, B], U8, isOutput=False)
    octx = nc.declare_dram_parameter("context", [T, B, D], F32, isOutput=True)
    oattn = nc.declare_dram_parameter("attn", [T, B, C], F32, isOutput=True)

    # DRAM bounce rows for the T-fold broadcast writes
    dctx = nc.dram_tensor("bounce_ctx", [B, D], F32)
    dattn = nc.dram_tensor("bounce_attn", [B, C], F32)

    with tile.TileContext(nc, trace_sim=trace_sim) as tc:
        with (
            tc.tile_pool(name="singles", bufs=1) as singles,
            tc.tile_pool(name="wnatp", bufs=2) as wnatp,
            tc.tile_pool(name="cpool", bufs=20) as cpool,
            tc.tile_pool(name="spool", bufs=2) as spool,
            tc.tile_pool(name="psum", bufs=1, space="PSUM") as psum,
        ):
            # ---------- phase 0: constants, W^T, h, proj ----------
            ident = singles.tile([128, 128], F32)
            make_identity(nc, ident)
            ones_row = singles.tile([1, 128], F32)
            nc.vector.memset(ones_row, 1.0)
            neg_tile = singles.tile([128, K], F32)
            nc.vector.memset(neg_tile, -1e30)

            # mask, whole shard in one DMA: [p, k, b]
            mask_sb = singles.tile([128, K, B], U8)
            nc.sync.dma_start(
                out=mask_sb[:], in_=maskp.rearrange("(k p) b -> p k b", p=128)
            )

            # h in interleaved layout: hall[p, b, f] holds h_b[p*HT + f]
            hall = singles.tile([128, B, HT], F32)
            for b in range(B):
                nc.sync.dma_start(
                    out=hall[:, b, :],
                    in_=hid[0:1, b, :],
                )

            # W^T tiles: wt[f][r, dout] = W[dout, HT*r + f]
            wt = []
            for f in range(HT):
                wt.append(singles.tile([128, D], F32, tag=f"wt{f}", name=f"wt{f}"))
            for dt_ in range(DT):
                wn = wnatp.tile([128, H], F32, tag="wnat")
                nc.sync.dma_start(out=wn[:], in_=Wp[dt_ * 128 : (dt_ + 1) * 128, :])
                for f in range(HT):
                    ps = psum.tile([128, 128], F32, tag="wtps", bufs=1)
                    # strided column view: h = f, f+HT, f+2*HT, ... (128 cols)
                    nc.tensor.transpose(ps[:], wn[:, f::HT], ident[:])
                    nc.scalar.copy(
                        out=wt[f][:, dt_ * 128 : (dt_ + 1) * 128], in_=ps[:]
                    )

            # proj per batch: psum [1, D] at partition 0, then broadcast
            pb = []
            for b in range(B):
                ps_proj = psum.tile([1, D], F32, tag="pswide", bufs=2, name="ps_proj")
                for ch in range(NCH):
                    for f in range(HT):
                        nc.tensor.matmul(
                            ps_proj[:, ch * 512 : (ch + 1) * 512],
                            lhsT=hall[:, b : b + 1, f],
                            rhs=wt[f][:, ch * 512 : (ch + 1) * 512],
                            start=(f == 0),
                            stop=(f == HT - 1),
                        )
                prow = spool.tile([1, D], F32, tag="projrow")
                nc.scalar.copy(prow[:], ps_proj[:])
                ps_pb = psum.tile([128, D], F32, tag="pswide", bufs=2, name="ps_pb")
                for ch in range(NCH):
                    nc.tensor.matmul(
                        ps_pb[:, ch * 512 : (ch + 1) * 512],
                        lhsT=ones_row[:],
                        rhs=prow[:, ch * 512 : (ch + 1) * 512],
                        start=True, stop=True,
                    )
                pb.append(singles.tile([128, D], F32, tag=f"pb{b}", name=f"pb{b}"))
                nc.scalar.copy(pb[b][:], ps_pb[:])

            # ---------- per-batch main loop ----------
            for b in range(B):
                # load C_b: K tiles [128, D] in natural layout
                cbk = []
                for k in range(K):
                    cb = cpool.tile([128, D], F32, tag="cb")
                    nc.sync.dma_start(
                        out=cb[:], in_=cvec[k * 128 : (k + 1) * 128, b, :]
                    )
                    cbk.append(cb)

                # scores: fused multiply + free-dim reduce on DVE
                scores = spool.tile([128, K], F32, tag="scores")
                scratch = spool.tile([128, D], F32, tag="ttscratch")
                for k in range(K):
                    nc.vector.scalar_tensor_tensor(
                        out=scratch[:],
                        in0=cbk[k][:],
                        scalar=0.0,
                        in1=pb[b][:],
                        op0=mybir.AluOpType.bypass,
                        op1=mybir.AluOpType.mult,
                        accum_out=scores[:, k : k + 1],
                    )

                # padding mask -> -1e30
                nc.vector.copy_predicated(
                    out=scores[:], mask=mask_sb[:, :, b], data=neg_tile[:]
                )

                # softmax over all C entries (partitions x K)
                rmax = spool.tile([128, 1], F32, tag="rmax")
                nc.vector.tensor_reduce(
                    out=rmax[:], in_=scores[:], axis=mybir.AxisListType.X,
                    op=mybir.AluOpType.max,
                )
                ps_t1 = psum.tile([1, 128], F32, tag="psmall", bufs=2, name="ps_t1")
                nc.tensor.transpose(ps_t1[:], rmax[:], ident[:])
                gmax = spool.tile([1, 1], F32, tag="gmax")
                nc.vector.tensor_reduce(
                    out=gmax[:], in_=ps_t1[:], axis=mybir.AxisListType.X,
                    op=mybir.AluOpType.max,
                )
                ps_b1 = psum.tile([128, 1], F32, tag="psmall", bufs=2, name="ps_b1")
                nc.tensor.matmul(ps_b1[:], lhsT=ones_row[:], rhs=gmax[:],
                                 start=True, stop=True)
                nmax = spool.tile([128, 1], F32, tag="nmax")
                nc.scalar.mul(nmax[:], ps_b1[:], -1.0)

                e_sb = spool.tile([128, K], F32, tag="esb")
                rsum = spool.tile([128, 1], F32, tag="rsum")
                nc.scalar.activation(
                    out=e_sb[:], in_=scores[:],
                    func=mybir.ActivationFunctionType.Exp,
                    bias=nmax[:], scale=1.0, accum_out=rsum[:],
                )
                ps_t2 = psum.tile([1, 128], F32, tag="psmall", bufs=2, name="ps_t2")
                nc.tensor.transpose(ps_t2[:], rsum[:], ident[:])
                gsum = spool.tile([1, 1], F32, tag="gsum")
                nc.vector.tensor_reduce(
                    out=gsum[:], in_=ps_t2[:], axis=mybir.AxisListType.X,
                    op=mybir.AluOpType.add,
                )
                rrow = spool.tile([1, 1], F32, tag="rrow")
                nc.vector.reciprocal(rrow[:], gsum[:])
                ps_b2 = psum.tile([128, 1], F32, tag="psmall", bufs=2, name="ps_b2")
                nc.tensor.matmul(ps_b2[:], lhsT=ones_row[:], rhs=rrow[:],
                                 start=True, stop=True)
                rtot = spool.tile([128, 1], F32, tag="rtot")
                nc.scalar.copy(rtot[:], ps_b2[:])

                w_sb = spool.tile([128, K], F32, tag="wsb")
                nc.vector.tensor_scalar_mul(w_sb[:], e_sb[:], rtot[:])

                # weighted sum of context vectors on PE (contract c)
                ps_ctx = psum.tile([1, D], F32, tag="pswide", bufs=2, name="ps_ctx")
                for ch in range(NCH):
                    for k in range(K):
                        nc.tensor.matmul(
                            ps_ctx[:, ch * 512 : (ch + 1) * 512],
                            lhsT=w_sb[:, k : k + 1],
                            rhs=cbk[k][:, ch * 512 : (ch + 1) * 512],
                            start=(k == 0),
                            stop=(k == K - 1),
                        )
                ctx_sb = spool.tile([1, D], F32, tag="ctxsb")
                nc.scalar.copy(ctx_sb[:], ps_ctx[:])

                # attn row layout: transpose w [128,K] -> [K,128]
                ps_wt = psum.tile([K, 128], F32, tag="pswt", bufs=1)
                nc.tensor.transpose(ps_wt[:], w_sb[:], ident[:])
                wrow = spool.tile([K, 128], F32, tag="wrow")
                nc.scalar.copy(wrow[:], ps_wt[:])

                # bounce rows to DRAM, then broadcast T-fold with step-0 source
                nc.sync.dma_start(out=dctx[b, :], in_=ctx_sb[:])
                nc.sync.dma_start(
                    out=dattn.rearrange("b (k p) -> b k p", p=128)[b], in_=wrow[:]
                )
                cbase = dctx[b, :]
                bctx = bass.AP(
                    tensor=cbase.tensor, offset=cbase.offset,
                    ap=[[0, T]] + list(cbase.ap),
                )
                nc.sync.dma_start(out=octx[:, b, :], in_=bctx)
                abase = dattn[b, :]
                battn = bass.AP(
                    tensor=abase.tensor, offset=abase.offset,
                    ap=[[0, T]] + list(abase.ap),
                )
                nc.sync.dma_start(out=oattn[:, b, :], in_=battn)

    _split_excess_waits(nc)
    return nc


_NC_CACHE = {}


def _get_nc(T):
    if T not in _NC_CACHE:
        _NC_CACHE[T] = build_nc(T=T)
    return _NC_CACHE[T]


def kernel(seqlen, hidden, contextvects, W, padding_mask):
    T = int(np.asarray(seqlen))
    hidden = np.asarray(hidden, dtype=np.float32)
    contextvects = np.ascontiguousarray(np.asarray(contextvects, dtype=np.float32))
    W = np.ascontiguousarray(np.asarray(W, dtype=np.float32))
    mask_u8 = np.ascontiguousarray(
        np.asarray(padding_mask).astype(np.uint8, copy=False)
    )

    Bfull = hidden.shape[1]
    assert Bfull % N_CORES == 0
    Bc = Bfull // N_CORES

    nc = _get_nc(T)
    in_maps = []
    for i in range(N_CORES):
        sl = slice(i * Bc, (i + 1) * Bc)
        in_maps.append(
            {
                "hidden": np.ascontiguousarray(hidden[:, sl, :]),
                "contextvects": np.ascontiguousarray(contextvects[:, sl, :]),
                "W": W,
                "padding_mask": mask_u8[:, sl],
            }
        )
    res = run_bass_kernel_spmd(nc, in_maps, core_ids=list(range(N_CORES)))
    context = np.concatenate([r["context"] for r in res.results], axis=1)
    attn = np.concatenate([r["attn"] for r in res.results], axis=1)
    return context, attn


if __name__ == "__main__":
    # quick smoke test with random data (no reference available here)
    rng = np.random.default_rng(0)
    inputs = {
        "seqlen": np.int64(64),
        "hidden": rng.standard_normal((1, 32, 1024), dtype=np.float32),
        "contextvects": rng.standard_normal((2048, 32, 1024), dtype=np.float32),
        "W": (rng.standard_normal((1024, 1024), dtype=np.float32) / 32.0),
        "padding_mask": np.zeros((2048, 32), dtype=bool),
    }
    ctx, attn = kernel(**inputs)
    print("context", ctx.shape, ctx.dtype, "attn", attn.shape, attn.dtype)


# revision 10
# speedup vs baseline: 34165.8765x; 34165.8765x over previous
"""Trainium2 Bass kernel for nn_AttentionMechanism (sparse_attention).

Reference computation (per full batch B=32):
    h          = hidden[0]                         # [B, H]
    proj       = h @ W.T                           # [B, D]
    scores     = einsum('cbd,bd->cb', ctx, proj)   # [C, B]
    scores     = where(mask, -inf, scores)
    attn       = softmax(scores, axis=0)           # [C, B]
    ctxout     = einsum('cb,cbd->bd', attn, ctx)   # [B, D]
    context    = broadcast ctxout  -> [T, B, D]
    attentions = broadcast attn.T  -> [T, B, C]

Sharding: data-parallel over batch across 8 NeuronCores (4 batches/core),
W replicated.  Per-core kernel keeps contextvects in natural layout
[c(part), d(free)]:
  - scores: fused multiply+reduce on DVE (contract d along free dim)
  - softmax: DVE row-max -> GPSIMD partition max -> ACT fused exp+rowsum
             -> PE ones-matmul partition sum is replaced by GPSIMD add
  - weighted sum: PE matmuls (contract c = partition dim), M=1, PSUM acc
  - seqlen broadcast done by step-0 source DMA from a DRAM bounce row
"""

import os
import sys

for _p in ("/opt/trn_rl_repo", "/root/.axon_site/_ro/trn_rl_repo"):
    if os.path.isdir(_p) and _p not in sys.path:
        sys.path.insert(0, _p)

import numpy as np

import concourse.bass as bass
import concourse.tile as tile
from concourse import mybir
import concourse.bass_isa as bass_isa
from concourse.bass_utils import run_bass_kernel_spmd
from concourse.masks import make_identity

F32 = mybir.dt.float32
U8 = mybir.dt.uint8

N_CORES = 8


_MAX_WAITS = 1


def _split_excess_waits(nc):
    """This container's walrus rejects instructions carrying more than ~2 sem
    waits.  Move excess waits onto same-engine NOPs inserted just before the
    offending instruction (waits still happen-before it in program order)."""
    n_split = 0
    for f in nc.m.functions:
        for bb in f.blocks:
            new_insts = []
            for ins in bb.instructions:
                si = ins.sync_info
                if si is not None and si.on_wait and len(si.on_wait) > _MAX_WAITS:
                    waits = list(si.on_wait)
                    keep = waits[: _MAX_WAITS]
                    rest = waits[_MAX_WAITS:]
                    si.on_wait.clear()
                    for w in keep:
                        si.on_wait.append(w)
                    for j, w in enumerate(rest):
                        nop = mybir.InstNoOp(
                            name=f"{ins.name}-wsplit{j}", ins=[], outs=[]
                        )
                        nop.engine = ins.engine
                        nop.sync_info = mybir.SyncInfo(on_wait=[w], on_update=[])
                        new_insts.append(nop)
                        n_split += 1
                new_insts.append(ins)
            bb.instructions.clear()
            for i in new_insts:
                bb.instructions.append(i)
    return n_split


def build_nc(T=64, C=2048, D=1024, H=1024, B=4, trace_sim=False):
    """Build the per-core bass program (B = batches on this core)."""
    assert C % 128 == 0 and D % 512 == 0 and H % 1024 == 0
    K = C // 128          # context tiles (partition dim c)
    HT = H // 128         # h contraction tiles
    DT = D // 128         # dout tiles of W (natural layout)
    NCH = D // 512        # 512-wide output chunks

    nc = bass.Bass()
    hid = nc.declare_dram_parameter("hidden", [1, B, H], F32, isOutput=False)
    cvec = nc.declare_dram_parameter("contextvects", [C, B, D], F32, isOutput=False)
    Wp = nc.declare_dram_parameter("W", [D, H], F32, isOutput=False)
    maskp = nc.declare_dram_parameter("padding_mask", [C, B], U8, isOutput=False)
    octx = nc.declare_dram_parameter("context", [T, B, D], F32, isOutput=True)
    oattn = nc.declare_dram_parameter("attn", [T, B, C], F32, isOutput=True)

    # DRAM bounce rows for the T-fold broadcast writes
    dctx = nc.dram_tensor("bounce_ctx", [B, D], F32)
    dattn = nc.dram_tensor("bounce_attn", [B, C], F32)

    with tile.TileContext(nc, trace_sim=trace_sim) as tc:
        with (
            tc.tile_pool(name="singles", bufs=1) as singles,
            tc.tile_pool(name="wnatp", bufs=2) as wnatp,
            tc.tile_pool(name="cpool", bufs=20) as cpool,
            tc.tile_pool(name="spool", bufs=2) as spool,
            tc.tile_pool(name="psum", bufs=1, space="PSUM") as psum,
        ):
            # ---------- phase 0: constants, W^T, h, proj ----------
            ident = singles.tile([128, 128], F32)
            make_identity(nc, ident)
            ones_row = singles.tile([1, 128], F32)
            nc.vector.memset(ones_row, 1.0)
            neg_tile = singles.tile([128, K], F32)
            nc.vector.memset(neg_tile, -1e30)

            # mask, whole shard in one DMA: [p, k, b]
            mask_sb = singles.tile([128, K, B], U8)
            nc.sync.dma_start(
                out=mask_sb[:], in_=maskp.rearrange("(k p) b -> p k b", p=128)
            )

            # h in interleaved layout: hall[p, b, f] holds h_b[p*HT + f]
            hall = singles.tile([128, B, HT], F32)
            for b in range(B):
                nc.sync.dma_start(
                    out=hall[:, b, :],
                    in_=hid[0:1, b, :],
                )

            # W^T tiles: wt[f][r, dout] = W[dout, HT*r + f]
            wt = []
            for f in range(HT):
                wt.append(singles.tile([128, D], F32, tag=f"wt{f}", name=f"wt{f}"))
            for dt_ in range(DT):
                wn = wnatp.tile([128, H], F32, tag="wnat")
                nc.sync.dma_start(out=wn[:], in_=Wp[dt_ * 128 : (dt_ + 1) * 128, :])
                for f in range(HT):
                    ps = psum.tile([128, 128], F32, tag="wtps", bufs=1)
                    # strided column view: h = f, f+HT, f+2*HT, ... (128 cols)
                    nc.tensor.transpose(ps[:], wn[:, f::HT], ident[:])
                    nc.scalar.copy(
                        out=wt[f][:, dt_ * 128 : (dt_ + 1) * 128], in_=ps[:]
                    )

            # proj for all B batches in one M=B chain: psum [B, D]
            ps_proj = psum.tile([B, D], F32, tag="pswide", bufs=2, name="ps_proj")
            for ch in range(NCH):
                for f in range(HT):
                    nc.tensor.matmul(
                        ps_proj[:, ch * 512 : (ch + 1) * 512],
                        lhsT=hall[:, :, f],
                        rhs=wt[f][:, ch * 512 : (ch + 1) * 512],
                        start=(f == 0),
                        stop=(f == HT - 1),
                    )
            proj_sb = singles.tile([B, D], F32)
            nc.scalar.copy(proj_sb[:], ps_proj[:])

            # move row b to partition 0 (SBUF->SBUF DMA), then ones-matmul
            # broadcast to all 128 partitions
            pb = []
            for b in range(B):
                prow = spool.tile([1, D], F32, tag="projrow", name=f"prow{b}")
                nc.sync.dma_start(out=prow[:], in_=proj_sb[b : b + 1, :])
                ps_pb = psum.tile([128, D], F32, tag="pswide", bufs=2, name="ps_pb")
                for ch in range(NCH):
                    nc.tensor.matmul(
                        ps_pb[:, ch * 512 : (ch + 1) * 512],
                        lhsT=ones_row[:],
                        rhs=prow[:, ch * 512 : (ch + 1) * 512],
                        start=True, stop=True,
                    )
                pb.append(singles.tile([128, D], F32, tag=f"pb{b}", name=f"pb{b}"))
                nc.scalar.copy(pb[b][:], ps_pb[:])

            # ---------- per-batch main loop ----------
            for b in range(B):
                # load C_b: K tiles [128, D] in natural layout
                cbk = []
                for k in range(K):
                    cb = cpool.tile([128, D], F32, tag="cb")
                    nc.sync.dma_start(
                        out=cb[:], in_=cvec[k * 128 : (k + 1) * 128, b, :]
                    )
                    cbk.append(cb)

                # scores: fused multiply + free-dim reduce on DVE
                scores = spool.tile([128, K], F32, tag="scores")
                scratch = spool.tile([128, D], F32, tag="ttscratch")
                for k in range(K):
                    nc.vector.scalar_tensor_tensor(
                        out=scratch[:],
                        in0=cbk[k][:],
                        scalar=0.0,
                        in1=pb[b][:],
                        op0=mybir.AluOpType.bypass,
                        op1=mybir.AluOpType.mult,
                        accum_out=scores[:, k : k + 1],
                    )

                # padding mask -> -1e30
                nc.vector.copy_predicated(
                    out=scores[:], mask=mask_sb[:, :, b], data=neg_tile[:]
                )

                # softmax over all C entries (partitions x K)
                rmax = spool.tile([128, 1], F32, tag="rmax")
                nc.vector.tensor_reduce(
                    out=rmax[:], in_=scores[:], axis=mybir.AxisListType.X,
                    op=mybir.AluOpType.max,
                )
                ps_t1 = psum.tile([1, 128], F32, tag="psmall", bufs=2, name="ps_t1")
                nc.tensor.transpose(ps_t1[:], rmax[:], ident[:])
                gmax = spool.tile([1, 1], F32, tag="gmax")
                nc.vector.tensor_reduce(
                    out=gmax[:], in_=ps_t1[:], axis=mybir.AxisListType.X,
                    op=mybir.AluOpType.max,
                )
                ps_b1 = psum.tile([128, 1], F32, tag="psmall", bufs=2, name="ps_b1")
                nc.tensor.matmul(ps_b1[:], lhsT=ones_row[:], rhs=gmax[:],
                                 start=True, stop=True)
                nmax = spool.tile([128, 1], F32, tag="nmax")
                nc.scalar.mul(nmax[:], ps_b1[:], -1.0)

                e_sb = spool.tile([128, K], F32, tag="esb")
                rsum = spool.tile([128, 1], F32, tag="rsum")
                nc.scalar.activation(
                    out=e_sb[:], in_=scores[:],
                    func=mybir.ActivationFunctionType.Exp,
                    bias=nmax[:], scale=1.0, accum_out=rsum[:],
                )
                ps_t2 = psum.tile([1, 128], F32, tag="psmall", bufs=2, name="ps_t2")
                nc.tensor.transpose(ps_t2[:], rsum[:], ident[:])
                gsum = spool.tile([1, 1], F32, tag="gsum")
                nc.vector.tensor_reduce(
                    out=gsum[:], in_=ps_t2[:], axis=mybir.AxisListType.X,
                    op=mybir.AluOpType.add,
                )
                rrow = spool.tile([1, 1], F32, tag="rrow")
                nc.vector.reciprocal(rrow[:], gsum[:])
                ps_b2 = psum.tile([128, 1], F32, tag="psmall", bufs=2, name="ps_b2")
                nc.tensor.matmul(ps_b2[:], lhsT=ones_row[:], rhs=rrow[:],
                                 start=True, stop=True)
                rtot = spool.tile([128, 1], F32, tag="rtot")
                nc.scalar.copy(rtot[:], ps_b2[:])

                w_sb = spool.tile([128, K], F32, tag="wsb")
                nc.vector.tensor_scalar_mul(w_sb[:], e_sb[:], rtot[:])

                # weighted sum of context vectors on PE (contract c)
                ps_ctx = psum.tile([1, D], F32, tag="pswide", bufs=2, name="ps_ctx")
                for ch in range(NCH):
                    for k in range(K):
                        nc.tensor.matmul(
                            ps_ctx[:, ch * 512 : (ch + 1) * 512],
                            lhsT=w_sb[:, k : k + 1],
                            rhs=cbk[k][:, ch * 512 : (ch + 1) * 512],
                            start=(k == 0),
                            stop=(k == K - 1),
                        )
                ctx_sb = spool.tile([1, D], F32, tag="ctxsb")
                nc.scalar.copy(ctx_sb[:], ps_ctx[:])

                # attn row layout: transpose w [128,K] -> [K,128]
                ps_wt = psum.tile([K, 128], F32, tag="pswt", bufs=1)
                nc.tensor.transpose(ps_wt[:], w_sb[:], ident[:])
                wrow = spool.tile([K, 128], F32, tag="wrow")
                nc.scalar.copy(wrow[:], ps_wt[:])

                # bounce rows to DRAM, then broadcast T-fold with step-0 source
                nc.sync.dma_start(out=dctx[b, :], in_=ctx_sb[:])
                nc.sync.dma_start(
                    out=dattn.rearrange("b (k p) -> b k p", p=128)[b], in_=wrow[:]
                )
                cbase = dctx[b, :]
                bctx = bass.AP(
                    tensor=cbase.tensor, offset=cbase.offset,
                    ap=[[0, T]] + list(cbase.ap),
                )
                nc.sync.dma_start(out=octx[:, b, :], in_=bctx)
                abase = dattn[b, :]
                battn = bass.AP(
                    tensor=abase.tensor, offset=abase.offset,
                    ap=[[0, T]] + list(abase.ap),
                )
                nc.sync.dma_start(out=oattn[:, b, :], in_=battn)

    _split_excess_waits(nc)
    return nc


_NC_CACHE = {}


def _get_nc(T):
    if T not in _NC_CACHE:
        _NC_CACHE[T] = build_nc(T=T)
    return _NC_CACHE[T]


def kernel(seqlen, hidden, contextvects, W, padding_mask):
    T = int(np.asarray(seqlen))
    hidden = np.asarray(hidden, dtype=np.float32)
    contextvects = np.ascontiguousarray(np.asarray(contextvects, dtype=np.float32))
    W = np.ascontiguousarray(np.asarray(W, dtype=np.float32))
    mask_u8 = np.ascontiguousarray(
        np.asarray(padding_mask).astype(np.uint8, copy=False)
    )

    Bfull = hidden.shape[1]
    assert Bfull % N_CORES == 0
    Bc = Bfull // N_CORES

    nc = _get_nc(T)
    in_maps = []
    for i in range(N_CORES):
        sl = slice(i * Bc, (i + 1) * Bc)
        in_maps.append(
            {
                "hidden": np.ascontiguousarray(hidden[:, sl, :]),
                "contextvects": np.ascontiguousarray(contextvects[:, sl, :]),
                "W": W,
                "padding_mask": mask_u8[:, sl],
            }
        )
    res = run_bass_kernel_spmd(nc, in_maps, core_ids=list(range(N_CORES)))
    context = np.concatenate([r["context"] for r in res.results], axis=1)
    attn = np.concatenate([r["attn"] for r in res.results], axis=1)
    return context, attn


if __name__ == "__main__":
    # quick smoke test with random data (no reference available here)
    rng = np.random.default_rng(0)
    inputs = {
        "seqlen": np.int64(64),
        "hidden": rng.standard_normal((1, 32, 1024), dtype=np.float32),
        "contextvects": rng.standard_normal((2048, 32, 1024), dtype=np.float32),
        "W": (rng.standard_normal((1024, 1024), dtype=np.float32) / 32.0),
        "padding_mask": np.zeros((2048, 32), dtype=bool),
    }
    ctx, attn = kernel(**inputs)
    print("context", ctx.shape, ctx.dtype, "attn", attn.shape, attn.dtype)


# revision 11
# speedup vs baseline: 39557.5872x; 1.1578x over previous
"""Trainium2 Bass kernel for nn_AttentionMechanism (sparse_attention).

Reference computation (per full batch B=32):
    h          = hidden[0]                         # [B, H]
    proj       = h @ W.T                           # [B, D]
    scores     = einsum('cbd,bd->cb', ctx, proj)   # [C, B]
    scores     = where(mask, -inf, scores)
    attn       = softmax(scores, axis=0)           # [C, B]
    ctxout     = einsum('cb,cbd->bd', attn, ctx)   # [B, D]
    context    = broadcast ctxout  -> [T, B, D]
    attentions = broadcast attn.T  -> [T, B, C]

Sharding: data-parallel over batch across 8 NeuronCores (4 batches/core),
W replicated.  Per-core kernel keeps contextvects in natural layout
[c(part), d(free)]:
  - scores: fused multiply+reduce on DVE (contract d along free dim)
  - softmax: DVE row-max -> GPSIMD partition max -> ACT fused exp+rowsum
             -> PE ones-matmul partition sum is replaced by GPSIMD add
  - weighted sum: PE matmuls (contract c = partition dim), M=1, PSUM acc
  - seqlen broadcast done by step-0 source DMA from a DRAM bounce row
"""

import os
import sys

for _p in ("/opt/trn_rl_repo", "/root/.axon_site/_ro/trn_rl_repo"):
    if os.path.isdir(_p) and _p not in sys.path:
        sys.path.insert(0, _p)

import numpy as np

import concourse.bass as bass
import concourse.tile as tile
from concourse import mybir
import concourse.bass_isa as bass_isa
from concourse.bass_utils import run_bass_kernel_spmd
from concourse.masks import make_identity

F32 = mybir.dt.float32
U8 = mybir.dt.uint8

N_CORES = 8


_MAX_WAITS = 1


def _split_excess_waits(nc):
    """This container's walrus rejects instructions carrying more than ~2 sem
    waits.  Move excess waits onto same-engine NOPs inserted just before the
    offending instruction (waits still happen-before it in program order)."""
    n_split = 0
    for f in nc.m.functions:
        for bb in f.blocks:
            new_insts = []
            for ins in bb.instructions:
                si = ins.sync_info
                if si is not None and si.on_wait and len(si.on_wait) > _MAX_WAITS:
                    waits = list(si.on_wait)
                    keep = waits[: _MAX_WAITS]
                    rest = waits[_MAX_WAITS:]
                    si.on_wait.clear()
                    for w in keep:
                        si.on_wait.append(w)
                    for j, w in enumerate(rest):
                        nop = mybir.InstNoOp(
                            name=f"{ins.name}-wsplit{j}", ins=[], outs=[]
                        )
                        nop.engine = ins.engine
                        nop.sync_info = mybir.SyncInfo(on_wait=[w], on_update=[])
                        new_insts.append(nop)
                        n_split += 1
                new_insts.append(ins)
            bb.instructions.clear()
            for i in new_insts:
                bb.instructions.append(i)
    return n_split


def build_nc(T=64, C=2048, D=1024, H=1024, B=4, trace_sim=False):
    """Build the per-core bass program (B = batches on this core)."""
    assert C % 128 == 0 and D % 512 == 0 and H % 1024 == 0
    K = C // 128          # context tiles (partition dim c)
    HT = H // 128         # h contraction tiles
    DT = D // 128         # dout tiles of W (natural layout)
    NCH = D // 512        # 512-wide output chunks

    nc = bass.Bass()
    hid = nc.declare_dram_parameter("hidden", [1, B, H], F32, isOutput=False)
    cvec = nc.declare_dram_parameter("contextvects", [C, B, D], F32, isOutput=False)
    Wp = nc.declare_dram_parameter("W", [D, H], F32, isOutput=False)
    maskp = nc.declare_dram_parameter("padding_mask", [C, B], U8, isOutput=False)
    octx = nc.declare_dram_parameter("context", [T, B, D], F32, isOutput=True)
    oattn = nc.declare_dram_parameter("attn", [T, B, C], F32, isOutput=True)

    # DRAM bounce rows for the T-fold broadcast writes
    dctx = nc.dram_tensor("bounce_ctx", [B, D], F32)
    dattn = nc.dram_tensor("bounce_attn", [B, C], F32)

    with tile.TileContext(nc, trace_sim=trace_sim) as tc:
        with (
            tc.tile_pool(name="singles", bufs=1) as singles,
            tc.tile_pool(name="wnatp", bufs=2) as wnatp,
            tc.tile_pool(name="cpool", bufs=28) as cpool,
            tc.tile_pool(name="spool", bufs=2) as spool,
            tc.tile_pool(name="psum", bufs=1, space="PSUM") as psum,
        ):
            # ---------- phase 0: constants, W^T, h, proj ----------
            ident = singles.tile([128, 128], F32)
            make_identity(nc, ident)
            ones_row = singles.tile([1, 128], F32)
            nc.vector.memset(ones_row, 1.0)
            neg_tile = singles.tile([128, K], F32)
            nc.vector.memset(neg_tile, -1e30)

            # mask, whole shard in one DMA: [p, k, b]
            mask_sb = singles.tile([128, K, B], U8)
            nc.sync.dma_start(
                out=mask_sb[:], in_=maskp.rearrange("(k p) b -> p k b", p=128)
            )

            # h in interleaved layout: hall[p, b, f] holds h_b[p*HT + f]
            hall = singles.tile([128, B, HT], F32)
            for b in range(B):
                nc.sync.dma_start(
                    out=hall[:, b, :],
                    in_=hid[0:1, b, :],
                )

            # W^T tiles: wt[f][r, dout] = W[dout, HT*r + f]
            wt = []
            for f in range(HT):
                wt.append(singles.tile([128, D], F32, tag=f"wt{f}", name=f"wt{f}"))
            for dt_ in range(DT):
                wn = wnatp.tile([128, H], F32, tag="wnat")
                nc.sync.dma_start(out=wn[:], in_=Wp[dt_ * 128 : (dt_ + 1) * 128, :])
                for f in range(HT):
                    ps = psum.tile([128, 128], F32, tag="wtps", bufs=1)
                    # strided column view: h = f, f+HT, f+2*HT, ... (128 cols)
                    nc.tensor.transpose(ps[:], wn[:, f::HT], ident[:])
                    dst = wt[f][:, dt_ * 128 : (dt_ + 1) * 128]
                    if (dt_ + f) % 2 == 0:
                        nc.scalar.copy(out=dst, in_=ps[:])
                    else:
                        nc.vector.tensor_copy(out=dst, in_=ps[:])

            # proj for all B batches in one M=B chain: psum [B, D]
            ps_proj = psum.tile([B, D], F32, tag="pswide", bufs=2, name="ps_proj")
            for ch in range(NCH):
                for f in range(HT):
                    nc.tensor.matmul(
                        ps_proj[:, ch * 512 : (ch + 1) * 512],
                        lhsT=hall[:, :, f],
                        rhs=wt[f][:, ch * 512 : (ch + 1) * 512],
                        start=(f == 0),
                        stop=(f == HT - 1),
                    )
            proj_sb = singles.tile([B, D], F32)
            nc.scalar.copy(proj_sb[:], ps_proj[:])

            # move row b to partition 0 (SBUF->SBUF DMA), then ones-matmul
            # broadcast to all 128 partitions
            pb = []
            for b in range(B):
                prow = spool.tile([1, D], F32, tag="projrow", name=f"prow{b}")
                nc.sync.dma_start(out=prow[:], in_=proj_sb[b : b + 1, :])
                ps_pb = psum.tile([128, D], F32, tag="pswide", bufs=2, name="ps_pb")
                for ch in range(NCH):
                    nc.tensor.matmul(
                        ps_pb[:, ch * 512 : (ch + 1) * 512],
                        lhsT=ones_row[:],
                        rhs=prow[:, ch * 512 : (ch + 1) * 512],
                        start=True, stop=True,
                    )
                pb.append(singles.tile([128, D], F32, tag=f"pb{b}", name=f"pb{b}"))
                if b % 2 == 0:
                    nc.scalar.copy(pb[b][:], ps_pb[:])
                else:
                    nc.vector.tensor_copy(out=pb[b][:], in_=ps_pb[:])

            # ---------- per-batch main loop ----------
            for b in range(B):
                # load C_b: K tiles [128, D] in natural layout
                cbk = []
                for k in range(K):
                    cb = cpool.tile([128, D], F32, tag="cb")
                    nc.sync.dma_start(
                        out=cb[:], in_=cvec[k * 128 : (k + 1) * 128, b, :]
                    )
                    cbk.append(cb)

                # scores: fused multiply + free-dim reduce on DVE
                scores = spool.tile([128, K], F32, tag="scores")
                scratch = spool.tile([128, D], F32, tag="ttscratch")
                for k in range(K):
                    nc.vector.scalar_tensor_tensor(
                        out=scratch[:],
                        in0=cbk[k][:],
                        scalar=0.0,
                        in1=pb[b][:],
                        op0=mybir.AluOpType.bypass,
                        op1=mybir.AluOpType.mult,
                        accum_out=scores[:, k : k + 1],
                    )

                # padding mask -> -1e30
                nc.vector.copy_predicated(
                    out=scores[:], mask=mask_sb[:, :, b], data=neg_tile[:]
                )

                # softmax over all C entries (partitions x K)
                rmax = spool.tile([128, 1], F32, tag="rmax")
                nc.vector.tensor_reduce(
                    out=rmax[:], in_=scores[:], axis=mybir.AxisListType.X,
                    op=mybir.AluOpType.max,
                )
                ps_t1 = psum.tile([1, 128], F32, tag="psmall", bufs=2, name="ps_t1")
                nc.tensor.transpose(ps_t1[:], rmax[:], ident[:])
                gmax = spool.tile([1, 1], F32, tag="gmax")
                nc.vector.tensor_reduce(
                    out=gmax[:], in_=ps_t1[:], axis=mybir.AxisListType.X,
                    op=mybir.AluOpType.max,
                )
                ps_b1 = psum.tile([128, 1], F32, tag="psmall", bufs=2, name="ps_b1")
                nc.tensor.matmul(ps_b1[:], lhsT=ones_row[:], rhs=gmax[:],
                                 start=True, stop=True)
                nmax = spool.tile([128, 1], F32, tag="nmax")
                nc.scalar.mul(nmax[:], ps_b1[:], -1.0)

                e_sb = spool.tile([128, K], F32, tag="esb")
                rsum = spool.tile([128, 1], F32, tag="rsum")
                nc.scalar.activation(
                    out=e_sb[:], in_=scores[:],
                    func=mybir.ActivationFunctionType.Exp,
                    bias=nmax[:], scale=1.0, accum_out=rsum[:],
                )
                ps_t2 = psum.tile([1, 128], F32, tag="psmall", bufs=2, name="ps_t2")
                nc.tensor.transpose(ps_t2[:], rsum[:], ident[:])
                gsum = spool.tile([1, 1], F32, tag="gsum")
                nc.vector.tensor_reduce(
                    out=gsum[:], in_=ps_t2[:], axis=mybir.AxisListType.X,
                    op=mybir.AluOpType.add,
                )
                rrow = spool.tile([1, 1], F32, tag="rrow")
                nc.vector.reciprocal(rrow[:], gsum[:])
                ps_b2 = psum.tile([128, 1], F32, tag="psmall", bufs=2, name="ps_b2")
                nc.tensor.matmul(ps_b2[:], lhsT=ones_row[:], rhs=rrow[:],
                                 start=True, stop=True)
                rtot = spool.tile([128, 1], F32, tag="rtot")
                nc.scalar.copy(rtot[:], ps_b2[:])

                w_sb = spool.tile([128, K], F32, tag="wsb")
                nc.vector.tensor_scalar_mul(w_sb[:], e_sb[:], rtot[:])

                # weighted sum of context vectors on PE (contract c)
                ps_ctx = psum.tile([1, D], F32, tag="pswide", bufs=2, name="ps_ctx")
                for ch in range(NCH):
                    for k in range(K):
                        nc.tensor.matmul(
                            ps_ctx[:, ch * 512 : (ch + 1) * 512],
                            lhsT=w_sb[:, k : k + 1],
                            rhs=cbk[k][:, ch * 512 : (ch + 1) * 512],
                            start=(k == 0),
                            stop=(k == K - 1),
                        )
                ctx_sb = spool.tile([1, D], F32, tag="ctxsb")
                nc.scalar.copy(ctx_sb[:], ps_ctx[:])

                # attn row layout: transpose w [128,K] -> [K,128]
                ps_wt = psum.tile([K, 128], F32, tag="pswt", bufs=1)
                nc.tensor.transpose(ps_wt[:], w_sb[:], ident[:])
                wrow = spool.tile([K, 128], F32, tag="wrow")
                nc.scalar.copy(wrow[:], ps_wt[:])

                # bounce rows to DRAM, then broadcast T-fold with step-0 source
                nc.sync.dma_start(out=dctx[b, :], in_=ctx_sb[:])
                nc.sync.dma_start(
                    out=dattn.rearrange("b (k p) -> b k p", p=128)[b], in_=wrow[:]
                )
                cbase = dctx[b, :]
                bctx = bass.AP(
                    tensor=cbase.tensor, offset=cbase.offset,
                    ap=[[0, T]] + list(cbase.ap),
                )
                nc.sync.dma_start(out=octx[:, b, :], in_=bctx)
                abase = dattn[b, :]
                battn = bass.AP(
                    tensor=abase.tensor, offset=abase.offset,
                    ap=[[0, T]] + list(abase.ap),
                )
                nc.sync.dma_start(out=oattn[:, b, :], in_=battn)

    _split_excess_waits(nc)
    return nc


_NC_CACHE = {}


def _get_nc(T):
    if T not in _NC_CACHE:
        _NC_CACHE[T] = build_nc(T=T)
    return _NC_CACHE[T]


def kernel(seqlen, hidden, contextvects, W, padding_mask):
    T = int(np.asarray(seqlen))
    hidden = np.asarray(hidden, dtype=np.float32)
    contextvects = np.ascontiguousarray(np.asarray(contextvects, dtype=np.float32))
    W = np.ascontiguousarray(np.asarray(W, dtype=np.float32))
    mask_u8 = np.ascontiguousarray(
        np.asarray(padding_mask).astype(np.uint8, copy=False)
    )

    Bfull = hidden.shape[1]
    assert Bfull % N_CORES == 0
    Bc = Bfull // N_CORES

    nc = _get_nc(T)
    in_maps = []
    for i in range(N_CORES):
        sl = slice(i * Bc, (i + 1) * Bc)
        in_maps.append(
            {
                "hidden": np.ascontiguousarray(hidden[:, sl, :]),
                "contextvects": np.ascontiguousarray(contextvects[:, sl, :]),
                "W": W,
                "padding_mask": mask_u8[:, sl],
            }
        )
    res = run_bass_kernel_spmd(nc, in_maps, core_ids=list(range(N_CORES)))
    context = np.concatenate([r["context"] for r in res.results], axis=1)
    attn = np.concatenate([r["attn"] for r in res.results], axis=1)
    return context, attn


if __name__ == "__main__":
    # quick smoke test with random data (no reference available here)
    rng = np.random.default_rng(0)
    inputs = {
        "seqlen": np.int64(64),
        "hidden": rng.standard_normal((1, 32, 1024), dtype=np.float32),
        "contextvects": rng.standard_normal((2048, 32, 1024), dtype=np.float32),
        "W": (rng.standard_normal((1024, 1024), dtype=np.float32) / 32.0),
        "padding_mask": np.zeros((2048, 32), dtype=bool),
    }
    ctx, attn = kernel(**inputs)
    print("context", ctx.shape, ctx.dtype, "attn", attn.shape, attn.dtype)


# revision 16
# speedup vs baseline: 47039.0711x; 1.1891x over previous
"""Trainium2 Bass kernel for nn_AttentionMechanism (sparse_attention).

Reference computation (per full batch B=32):
    h          = hidden[0]                         # [B, H]
    proj       = h @ W.T                           # [B, D]
    scores     = einsum('cbd,bd->cb', ctx, proj)   # [C, B]
    scores     = where(mask, -inf, scores)
    attn       = softmax(scores, axis=0)           # [C, B]
    ctxout     = einsum('cb,cbd->bd', attn, ctx)   # [B, D]
    context    = broadcast ctxout  -> [T, B, D]
    attentions = broadcast attn.T  -> [T, B, C]

Sharding: data-parallel over batch across 8 NeuronCores (4 batches/core),
W replicated.  Per-core kernel keeps contextvects in natural layout
[c(part), d(free)]:
  - scores: fused multiply+reduce on DVE (contract d along free dim)
  - softmax: DVE row-max -> GPSIMD partition max -> ACT fused exp+rowsum
             -> PE ones-matmul partition sum is replaced by GPSIMD add
  - weighted sum: PE matmuls (contract c = partition dim), M=1, PSUM acc
  - seqlen broadcast done by step-0 source DMA from a DRAM bounce row
"""

import os
import sys

for _p in ("/opt/trn_rl_repo", "/root/.axon_site/_ro/trn_rl_repo"):
    if os.path.isdir(_p) and _p not in sys.path:
        sys.path.insert(0, _p)

import numpy as np

import concourse.bass as bass
import concourse.tile as tile
from concourse import mybir
import concourse.bass_isa as bass_isa
from concourse.bass_utils import run_bass_kernel_spmd
from concourse.masks import make_identity

F32 = mybir.dt.float32
F32R = mybir.dt.float32r
U8 = mybir.dt.uint8

N_CORES = 8


_MAX_WAITS = 1


def _split_excess_waits(nc):
    """This container's walrus rejects instructions carrying more than ~2 sem
    waits.  Move excess waits onto same-engine NOPs inserted just before the
    offending instruction (waits still happen-before it in program order)."""
    n_split = 0
    for f in nc.m.functions:
        for bb in f.blocks:
            new_insts = []
            for ins in bb.instructions:
                si = ins.sync_info
                if si is not None and si.on_wait and len(si.on_wait) > _MAX_WAITS:
                    waits = list(si.on_wait)
                    keep = waits[: _MAX_WAITS]
                    rest = waits[_MAX_WAITS:]
                    si.on_wait.clear()
                    for w in keep:
                        si.on_wait.append(w)
                    for j, w in enumerate(rest):
                        nop = mybir.InstNoOp(
                            name=f"{ins.name}-wsplit{j}", ins=[], outs=[]
                        )
                        nop.engine = ins.engine
                        nop.sync_info = mybir.SyncInfo(on_wait=[w], on_update=[])
                        new_insts.append(nop)
                        n_split += 1
                new_insts.append(ins)
            bb.instructions.clear()
            for i in new_insts:
                bb.instructions.append(i)
    return n_split


def build_nc(T=64, C=2048, D=1024, H=1024, B=4, trace_sim=False):
    """Build the per-core bass program (B = batches on this core)."""
    assert C % 128 == 0 and D % 512 == 0 and H % 1024 == 0
    K = C // 128          # context tiles (partition dim c)
    HT = H // 128         # h contraction tiles
    DT = D // 128         # dout tiles of W (natural layout)
    NCH = D // 512        # 512-wide output chunks

    nc = bass.Bass()
    hid = nc.declare_dram_parameter("hidden", [1, B, H], F32, isOutput=False)
    cvec = nc.declare_dram_parameter("contextvects", [C, B, D], F32, isOutput=False)
    Wp = nc.declare_dram_parameter("W", [D, H], F32, isOutput=False)
    maskp = nc.declare_dram_parameter("padding_mask", [C, B], U8, isOutput=False)
    octx = nc.declare_dram_parameter("context", [T, B, D], F32, isOutput=True)
    oattn = nc.declare_dram_parameter("attn", [T, B, C], F32, isOutput=True)

    # DRAM bounce rows for the T-fold broadcast writes
    dctx = nc.dram_tensor("bounce_ctx", [B, D], F32)
    dattn = nc.dram_tensor("bounce_attn", [B, C], F32)

    with tile.TileContext(nc, trace_sim=trace_sim) as tc:
        with (
            tc.tile_pool(name="singles", bufs=1) as singles,
            tc.tile_pool(name="cpool", bufs=8) as cpool,
            tc.tile_pool(name="spool", bufs=2) as spool,
            tc.tile_pool(name="psum", bufs=1, space="PSUM") as psum,
        ):
            # ---------- phase 0: constants, W^T, h, proj ----------
            ident = singles.tile([128, 128], F32)
            make_identity(nc, ident)
            ones_row = singles.tile([1, 128], F32)
            nc.vector.memset(ones_row, 1.0)
            neg_tile = singles.tile([128, K], F32)
            nc.vector.memset(neg_tile, -1e30)

            # mask, whole shard in one DMA: [p, k, b]
            mask_sb = singles.tile([128, K, B], U8)
            nc.sync.dma_start(
                out=mask_sb[:], in_=maskp.rearrange("(k p) b -> p k b", p=128)
            )

            # h in interleaved layout: hall[p, b, f] holds h_b[p*HT + f]
            hall = singles.tile([128, B, HT], F32)
            for b in range(B):
                nc.sync.dma_start(
                    out=hall[:, b, :],
                    in_=hid[0:1, b, :],
                )

            # W^T tiles: wt[f][r, dout] = W[dout, HT*r + f]
            wtp_cm = tc.tile_pool(name="wtp", bufs=1)
            wtp = wtp_cm.__enter__()
            wnatp_cm = tc.tile_pool(name="wnatp", bufs=2)
            wnatp = wnatp_cm.__enter__()
            wt = []
            for f in range(HT):
                wt.append(wtp.tile([128, D], F32, tag=f"wt{f}", name=f"wt{f}"))
            for dt_ in range(DT):
                wn = wnatp.tile([128, H], F32, tag="wnat")
                nc.sync.dma_start(out=wn[:], in_=Wp[dt_ * 128 : (dt_ + 1) * 128, :])
                for f in range(HT):
                    ps = psum.tile([128, 128], F32, tag="wtps", bufs=1)
                    # strided column view: h = f, f+HT, f+2*HT, ... (128 cols)
                    nc.tensor.transpose(ps[:], wn[:, f::HT], ident[:])
                    dst = wt[f][:, dt_ * 128 : (dt_ + 1) * 128]
                    if (dt_ + f) % 2 == 0:
                        nc.scalar.copy(out=dst, in_=ps[:])
                    else:
                        nc.vector.tensor_copy(out=dst, in_=ps[:])

            # proj for all B batches in one M=B chain: psum [B, D]
            ps_proj = psum.tile([B, D], F32, tag="pswide", bufs=2, name="ps_proj")
            for ch in range(NCH):
                for f in range(HT):
                    nc.tensor.matmul(
                        ps_proj[:, ch * 512 : (ch + 1) * 512],
                        lhsT=hall[:, :, f],
                        rhs=wt[f][:, ch * 512 : (ch + 1) * 512],
                        start=(f == 0),
                        stop=(f == HT - 1),
                    )
            proj_sb = singles.tile([B, D], F32)
            nc.scalar.copy(proj_sb[:], ps_proj[:])

            # move row b to partition 0 (SBUF->SBUF DMA), then ones-matmul
            # broadcast to all 128 partitions
            pb = []
            rproj = []
            for b in range(B):
                prow = spool.tile([1, D], F32, tag="projrow", name=f"prow{b}")
                nc.sync.dma_start(out=prow[:], in_=proj_sb[b : b + 1, :])
                rproj.append(
                    singles.tile([1, D], F32, tag=f"rproj{b}", name=f"rproj{b}")
                )
                nc.vector.reciprocal(rproj[b][:], prow[:])
                ps_pb = psum.tile([128, D], F32, tag="pswide", bufs=2, name="ps_pb")
                for ch in range(NCH):
                    nc.tensor.matmul(
                        ps_pb[:, ch * 512 : (ch + 1) * 512],
                        lhsT=ones_row[:],
                        rhs=prow[:, ch * 512 : (ch + 1) * 512],
                        start=True, stop=True,
                    )
                pb.append(singles.tile([128, D], F32, tag=f"pb{b}", name=f"pb{b}"))
                if b % 2 == 0:
                    nc.scalar.copy(pb[b][:], ps_pb[:])
                else:
                    nc.vector.tensor_copy(out=pb[b][:], in_=ps_pb[:])

            wnatp_cm.__exit__(None, None, None)
            wtp_cm.__exit__(None, None, None)
            ppool_cm = tc.tile_pool(name="ppool", bufs=18)
            ppool = ppool_cm.__enter__()

            # ---------- per-batch main loop ----------
            for b in range(B):
                # load C_b: K tiles [128, D] in natural layout
                cbk = []
                for k in range(K):
                    cb = cpool.tile([128, D], F32, tag="cb")
                    nc.sync.dma_start(
                        out=cb[:], in_=cvec[k * 128 : (k + 1) * 128, b, :]
                    )
                    cbk.append(cb)

                # scores: fused multiply + free-dim reduce on DVE.  The
                # elementwise product P[c,d] = C[c,d]*proj[d] is kept (in
                # f32r) as the weighted-sum input; ctx divides by proj later.
                scores = spool.tile([128, K], F32, tag="scores")
                pk = []
                for k in range(K):
                    pt = ppool.tile([128, D], F32R, tag="pk", name=f"pk{k}")
                    nc.vector.scalar_tensor_tensor(
                        out=pt[:],
                        in0=cbk[k][:],
                        scalar=0.0,
                        in1=pb[b][:],
                        op0=mybir.AluOpType.bypass,
                        op1=mybir.AluOpType.mult,
                        accum_out=scores[:, k : k + 1],
                    )
                    pk.append(pt)

                # padding mask -> -1e30
                nc.vector.copy_predicated(
                    out=scores[:], mask=mask_sb[:, :, b], data=neg_tile[:]
                )

                # softmax over all C entries (partitions x K)
                rmax = spool.tile([128, 1], F32, tag="rmax")
                nc.vector.tensor_reduce(
                    out=rmax[:], in_=scores[:], axis=mybir.AxisListType.X,
                    op=mybir.AluOpType.max,
                )
                ps_t1 = psum.tile([1, 128], F32, tag="psmall", bufs=2, name="ps_t1")
                nc.tensor.transpose(ps_t1[:], rmax[:], ident[:])
                gmax = spool.tile([1, 1], F32, tag="gmax")
                nc.vector.tensor_reduce(
                    out=gmax[:], in_=ps_t1[:], axis=mybir.AxisListType.X,
                    op=mybir.AluOpType.max,
                )
                ps_b1 = psum.tile([128, 1], F32, tag="psmall", bufs=2, name="ps_b1")
                nc.tensor.matmul(ps_b1[:], lhsT=ones_row[:], rhs=gmax[:],
                                 start=True, stop=True)
                nmax = spool.tile([128, 1], F32, tag="nmax")
                nc.scalar.mul(nmax[:], ps_b1[:], -1.0)

                e_sb = spool.tile([128, K], F32, tag="esb")
                rsum = spool.tile([128, 1], F32, tag="rsum")
                nc.scalar.activation(
                    out=e_sb[:], in_=scores[:],
                    func=mybir.ActivationFunctionType.Exp,
                    bias=nmax[:], scale=1.0, accum_out=rsum[:],
                )
                # weighted sum on PE in f32r (full-rate streaming), using the
                # P product tiles; UNNORMALIZED e as stationary weights.
                e_r = spool.tile([128, K], F32R, tag="er")
                nc.scalar.copy(e_r[:], e_sb[:])
                ps_ctx = psum.tile([1, D], F32, tag="pswide", bufs=2, name="ps_ctx")
                for ch in range(NCH):
                    for k in range(K):
                        nc.tensor.matmul(
                            ps_ctx[:, ch * 512 : (ch + 1) * 512],
                            lhsT=e_r[:, k : k + 1],
                            rhs=pk[k][:, ch * 512 : (ch + 1) * 512],
                            start=(k == 0),
                            stop=(k == K - 1),
                        )

                # total sum + reciprocal (off the wsum critical path)
                ps_t2 = psum.tile([1, 128], F32, tag="psmall", bufs=2, name="ps_t2")
                nc.tensor.transpose(ps_t2[:], rsum[:], ident[:])
                gsum = spool.tile([1, 1], F32, tag="gsum")
                nc.vector.tensor_reduce(
                    out=gsum[:], in_=ps_t2[:], axis=mybir.AxisListType.X,
                    op=mybir.AluOpType.add,
                )
                rrow = spool.tile([1, 1], F32, tag="rrow")
                nc.vector.reciprocal(rrow[:], gsum[:])
                ps_b2 = psum.tile([128, 1], F32, tag="psmall", bufs=2, name="ps_b2")
                nc.tensor.matmul(ps_b2[:], lhsT=ones_row[:], rhs=rrow[:],
                                 start=True, stop=True)
                rtot = spool.tile([128, 1], F32, tag="rtot")
                nc.scalar.copy(rtot[:], ps_b2[:])

                # normalized weights for the attentions output
                w_sb = spool.tile([128, K], F32, tag="wsb")
                nc.vector.tensor_scalar_mul(w_sb[:], e_sb[:], rtot[:])

                ctx_sb = spool.tile([1, D], F32, tag="ctxsb")
                nc.vector.scalar_tensor_tensor(
                    out=ctx_sb[:],
                    in0=ps_ctx[:],
                    scalar=rrow[:],
                    in1=rproj[b][:],
                    op0=mybir.AluOpType.mult,
                    op1=mybir.AluOpType.mult,
                )

                # attn row layout: transpose w [128,K] -> [K,128]
                ps_wt = psum.tile([K, 128], F32, tag="pswt", bufs=1)
                nc.tensor.transpose(ps_wt[:], w_sb[:], ident[:])
                wrow = spool.tile([K, 128], F32, tag="wrow")
                nc.scalar.copy(wrow[:], ps_wt[:])

                # bounce rows to DRAM, then broadcast T-fold with step-0 source
                nc.sync.dma_start(out=dctx[b, :], in_=ctx_sb[:])
                nc.sync.dma_start(
                    out=dattn.rearrange("b (k p) -> b k p", p=128)[b], in_=wrow[:]
                )
                cbase = dctx[b, :]
                bctx = bass.AP(
                    tensor=cbase.tensor, offset=cbase.offset,
                    ap=[[0, T]] + list(cbase.ap),
                )
                nc.sync.dma_start(out=octx[:, b, :], in_=bctx)
                abase = dattn[b, :]
                battn = bass.AP(
                    tensor=abase.tensor, offset=abase.offset,
                    ap=[[0, T]] + list(abase.ap),
                )
                nc.sync.dma_start(out=oattn[:, b, :], in_=battn)

            ppool_cm.__exit__(None, None, None)

    _split_excess_waits(nc)
    return nc


_NC_CACHE = {}


def _get_nc(T):
    if T not in _NC_CACHE:
        _NC_CACHE[T] = build_nc(T=T)
    return _NC_CACHE[T]


def kernel(seqlen, hidden, contextvects, W, padding_mask):
    T = int(np.asarray(seqlen))
    hidden = np.asarray(hidden, dtype=np.float32)
    contextvects = np.ascontiguousarray(np.asarray(contextvects, dtype=np.float32))
    W = np.ascontiguousarray(np.asarray(W, dtype=np.float32))
    mask_u8 = np.ascontiguousarray(
        np.asarray(padding_mask).astype(np.uint8, copy=False)
    )

    Bfull = hidden.shape[1]
    assert Bfull % N_CORES == 0
    Bc = Bfull // N_CORES

    nc = _get_nc(T)
    in_maps = []
    for i in range(N_CORES):
        sl = slice(i * Bc, (i + 1) * Bc)
        in_maps.append(
            {
                "hidden": np.ascontiguousarray(hidden[:, sl, :]),
                "contextvects": np.ascontiguousarray(contextvects[:, sl, :]),
                "W": W,
                "padding_mask": mask_u8[:, sl],
            }
        )
    res = run_bass_kernel_spmd(nc, in_maps, core_ids=list(range(N_CORES)))
    context = np.concatenate([r["context"] for r in res.results], axis=1)
    attn = np.concatenate([r["attn"] for r in res.results], axis=1)
    return context, attn


if __name__ == "__main__":
    # quick smoke test with random data (no reference available here)
    rng = np.random.default_rng(0)
    inputs = {
        "seqlen": np.int64(64),
        "hidden": rng.standard_normal((1, 32, 1024), dtype=np.float32),
        "contextvects": rng.standard_normal((2048, 32, 1024), dtype=np.float32),
        "W": (rng.standard_normal((1024, 1024), dtype=np.float32) / 32.0),
        "padding_mask": np.zeros((2048, 32), dtype=bool),
    }
    ctx, attn = kernel(**inputs)
    print("context", ctx.shape, ctx.dtype, "attn", attn.shape, attn.dtype)


# revision 17
# speedup vs baseline: 49770.1556x; 1.0581x over previous
"""Trainium2 Bass kernel for nn_AttentionMechanism (sparse_attention).

Reference computation (per full batch B=32):
    h          = hidden[0]                         # [B, H]
    proj       = h @ W.T                           # [B, D]
    scores     = einsum('cbd,bd->cb', ctx, proj)   # [C, B]
    scores     = where(mask, -inf, scores)
    attn       = softmax(scores, axis=0)           # [C, B]
    ctxout     = einsum('cb,cbd->bd', attn, ctx)   # [B, D]
    context    = broadcast ctxout  -> [T, B, D]
    attentions = broadcast attn.T  -> [T, B, C]

Sharding: data-parallel over batch across 8 NeuronCores (4 batches/core),
W replicated.  Per-core kernel keeps contextvects in natural layout
[c(part), d(free)]:
  - scores: fused multiply+reduce on DVE (contract d along free dim)
  - softmax: DVE row-max -> GPSIMD partition max -> ACT fused exp+rowsum
             -> PE ones-matmul partition sum is replaced by GPSIMD add
  - weighted sum: PE matmuls (contract c = partition dim), M=1, PSUM acc
  - seqlen broadcast done by step-0 source DMA from a DRAM bounce row
"""

import os
import sys

for _p in ("/opt/trn_rl_repo", "/root/.axon_site/_ro/trn_rl_repo"):
    if os.path.isdir(_p) and _p not in sys.path:
        sys.path.insert(0, _p)

import numpy as np

import concourse.bass as bass
import concourse.tile as tile
from concourse import mybir
import concourse.bass_isa as bass_isa
from concourse.bass_utils import run_bass_kernel_spmd
from concourse.masks import make_identity

F32 = mybir.dt.float32
F32R = mybir.dt.float32r
U8 = mybir.dt.uint8

N_CORES = 8


_MAX_WAITS = 1


def _split_excess_waits(nc):
    """This container's walrus rejects instructions carrying more than ~2 sem
    waits.  Move excess waits onto same-engine NOPs inserted just before the
    offending instruction (waits still happen-before it in program order)."""
    n_split = 0
    for f in nc.m.functions:
        for bb in f.blocks:
            new_insts = []
            for ins in bb.instructions:
                si = ins.sync_info
                if si is not None and si.on_wait and len(si.on_wait) > _MAX_WAITS:
                    waits = list(si.on_wait)
                    keep = waits[: _MAX_WAITS]
                    rest = waits[_MAX_WAITS:]
                    si.on_wait.clear()
                    for w in keep:
                        si.on_wait.append(w)
                    for j, w in enumerate(rest):
                        nop = mybir.InstNoOp(
                            name=f"{ins.name}-wsplit{j}", ins=[], outs=[]
                        )
                        nop.engine = ins.engine
                        nop.sync_info = mybir.SyncInfo(on_wait=[w], on_update=[])
                        new_insts.append(nop)
                        n_split += 1
                new_insts.append(ins)
            bb.instructions.clear()
            for i in new_insts:
                bb.instructions.append(i)
    return n_split


def build_nc(T=64, C=2048, D=1024, H=1024, B=4, trace_sim=False):
    """Build the per-core bass program (B = batches on this core)."""
    assert C % 128 == 0 and D % 512 == 0 and H % 1024 == 0
    K = C // 128          # context tiles (partition dim c)
    HT = H // 128         # h contraction tiles
    DT = D // 128         # dout tiles of W (natural layout)
    NCH = D // 512        # 512-wide output chunks

    nc = bass.Bass()
    hid = nc.declare_dram_parameter("hidden", [1, B, H], F32, isOutput=False)
    cvec = nc.declare_dram_parameter("contextvects", [C, B, D], F32, isOutput=False)
    Wp = nc.declare_dram_parameter("W", [D, H], F32, isOutput=False)
    maskp = nc.declare_dram_parameter("padding_mask", [C, B], U8, isOutput=False)
    octx = nc.declare_dram_parameter("context", [T, B, D], F32, isOutput=True)
    oattn = nc.declare_dram_parameter("attn", [T, B, C], F32, isOutput=True)

    # DRAM bounce rows for the T-fold broadcast writes
    dctx = nc.dram_tensor("bounce_ctx", [B, D], F32)
    dattn = nc.dram_tensor("bounce_attn", [B, C], F32)

    with tile.TileContext(nc, trace_sim=trace_sim) as tc:
        with (
            tc.tile_pool(name="singles", bufs=1) as singles,
            tc.tile_pool(name="cpool", bufs=8) as cpool,
            tc.tile_pool(name="spool", bufs=2) as spool,
            tc.tile_pool(name="psum", bufs=1, space="PSUM") as psum,
        ):
            # ---------- phase 0: constants, W^T, h, proj ----------
            ident = singles.tile([128, 128], F32)
            make_identity(nc, ident)
            ones_row = singles.tile([1, 128], F32)
            nc.vector.memset(ones_row, 1.0)
            neg_tile = singles.tile([128, K], F32)
            nc.vector.memset(neg_tile, -1e30)

            # mask, whole shard in one DMA: [p, k, b]
            mask_sb = singles.tile([128, K, B], U8)
            nc.sync.dma_start(
                out=mask_sb[:], in_=maskp.rearrange("(k p) b -> p k b", p=128)
            )

            # h in interleaved layout: hall[p, b, f] holds h_b[p*HT + f]
            hall = singles.tile([128, B, HT], F32)
            for b in range(B):
                nc.sync.dma_start(
                    out=hall[:, b, :],
                    in_=hid[0:1, b, :],
                )

            # W^T tiles: wt[f][r, dout] = W[dout, HT*r + f]
            wtp_cm = tc.tile_pool(name="wtp", bufs=1)
            wtp = wtp_cm.__enter__()
            wnatp_cm = tc.tile_pool(name="wnatp", bufs=2)
            wnatp = wnatp_cm.__enter__()
            wt = []
            for f in range(HT):
                wt.append(wtp.tile([128, D], F32, tag=f"wt{f}", name=f"wt{f}"))
            for dt_ in range(DT):
                wn = wnatp.tile([128, H], F32, tag="wnat")
                nc.sync.dma_start(out=wn[:], in_=Wp[dt_ * 128 : (dt_ + 1) * 128, :])
                for f in range(HT):
                    ps = psum.tile([128, 128], F32, tag="wtps", bufs=1)
                    # strided column view: h = f, f+HT, f+2*HT, ... (128 cols)
                    nc.tensor.transpose(ps[:], wn[:, f::HT], ident[:])
                    dst = wt[f][:, dt_ * 128 : (dt_ + 1) * 128]
                    if (dt_ + f) % 2 == 0:
                        nc.scalar.copy(out=dst, in_=ps[:])
                    else:
                        nc.vector.tensor_copy(out=dst, in_=ps[:])

            # proj for all B batches in one M=B chain: psum [B, D]
            ps_proj = psum.tile([B, D], F32, tag="pswide", bufs=2, name="ps_proj")
            for ch in range(NCH):
                for f in range(HT):
                    nc.tensor.matmul(
                        ps_proj[:, ch * 512 : (ch + 1) * 512],
                        lhsT=hall[:, :, f],
                        rhs=wt[f][:, ch * 512 : (ch + 1) * 512],
                        start=(f == 0),
                        stop=(f == HT - 1),
                    )
            proj_sb = singles.tile([B, D], F32)
            nc.scalar.copy(proj_sb[:], ps_proj[:])

            # move row b to partition 0 (SBUF->SBUF DMA), then ones-matmul
            # broadcast to all 128 partitions
            pb = []
            rproj = []
            for b in range(B):
                prow = spool.tile([1, D], F32, tag="projrow", name=f"prow{b}")
                nc.sync.dma_start(out=prow[:], in_=proj_sb[b : b + 1, :])
                rproj.append(
                    singles.tile([1, D], F32, tag=f"rproj{b}", name=f"rproj{b}")
                )
                nc.vector.reciprocal(rproj[b][:], prow[:])
                ps_pb = psum.tile([128, D], F32, tag="pswide", bufs=2, name="ps_pb")
                for ch in range(NCH):
                    nc.tensor.matmul(
                        ps_pb[:, ch * 512 : (ch + 1) * 512],
                        lhsT=ones_row[:],
                        rhs=prow[:, ch * 512 : (ch + 1) * 512],
                        start=True, stop=True,
                    )
                pb.append(singles.tile([128, D], F32, tag=f"pb{b}", name=f"pb{b}"))
                if b % 2 == 0:
                    nc.scalar.copy(pb[b][:], ps_pb[:])
                else:
                    nc.vector.tensor_copy(out=pb[b][:], in_=ps_pb[:])

            wnatp_cm.__exit__(None, None, None)
            wtp_cm.__exit__(None, None, None)
            ppool_cm = tc.tile_pool(name="ppool", bufs=18)
            ppool = ppool_cm.__enter__()

            # ---------- per-batch main loop ----------
            for b in range(B):
                # load C_b: K tiles [128, D] in natural layout
                cbk = []
                for k in range(K):
                    cb = cpool.tile([128, D], F32, tag="cb")
                    eng = nc.sync if k % 2 == 0 else nc.scalar
                    eng.dma_start(
                        out=cb[:], in_=cvec[k * 128 : (k + 1) * 128, b, :]
                    )
                    cbk.append(cb)

                # scores: fused multiply + free-dim reduce on DVE.  The
                # elementwise product P[c,d] = C[c,d]*proj[d] is kept (in
                # f32r) as the weighted-sum input; ctx divides by proj later.
                scores = spool.tile([128, K], F32, tag="scores")
                pk = []
                for k in range(K):
                    pt = ppool.tile([128, D], F32R, tag="pk", name=f"pk{k}")
                    nc.vector.scalar_tensor_tensor(
                        out=pt[:],
                        in0=cbk[k][:],
                        scalar=0.0,
                        in1=pb[b][:],
                        op0=mybir.AluOpType.bypass,
                        op1=mybir.AluOpType.mult,
                        accum_out=scores[:, k : k + 1],
                    )
                    pk.append(pt)

                # padding mask -> -1e30
                nc.vector.copy_predicated(
                    out=scores[:], mask=mask_sb[:, :, b], data=neg_tile[:]
                )

                # softmax over all C entries (partitions x K)
                rmax = spool.tile([128, 1], F32, tag="rmax")
                nc.vector.tensor_reduce(
                    out=rmax[:], in_=scores[:], axis=mybir.AxisListType.X,
                    op=mybir.AluOpType.max,
                )
                ps_t1 = psum.tile([1, 128], F32, tag="psmall", bufs=2, name="ps_t1")
                nc.tensor.transpose(ps_t1[:], rmax[:], ident[:])
                gmax = spool.tile([1, 1], F32, tag="gmax")
                nc.vector.tensor_reduce(
                    out=gmax[:], in_=ps_t1[:], axis=mybir.AxisListType.X,
                    op=mybir.AluOpType.max,
                )
                ps_b1 = psum.tile([128, 1], F32, tag="psmall", bufs=2, name="ps_b1")
                nc.tensor.matmul(ps_b1[:], lhsT=ones_row[:], rhs=gmax[:],
                                 start=True, stop=True)
                nmax = spool.tile([128, 1], F32, tag="nmax")
                nc.scalar.mul(nmax[:], ps_b1[:], -1.0)

                e_sb = spool.tile([128, K], F32, tag="esb")
                rsum = spool.tile([128, 1], F32, tag="rsum")
                nc.scalar.activation(
                    out=e_sb[:], in_=scores[:],
                    func=mybir.ActivationFunctionType.Exp,
                    bias=nmax[:], scale=1.0, accum_out=rsum[:],
                )
                # weighted sum on PE in f32r (full-rate streaming), using the
                # P product tiles; UNNORMALIZED e as stationary weights.
                e_r = spool.tile([128, K], F32R, tag="er")
                nc.scalar.copy(e_r[:], e_sb[:])
                ps_ctx = psum.tile([1, D], F32, tag="pswide", bufs=2, name="ps_ctx")
                for ch in range(NCH):
                    for k in range(K):
                        nc.tensor.matmul(
                            ps_ctx[:, ch * 512 : (ch + 1) * 512],
                            lhsT=e_r[:, k : k + 1],
                            rhs=pk[k][:, ch * 512 : (ch + 1) * 512],
                            start=(k == 0),
                            stop=(k == K - 1),
                        )

                # total sum + reciprocal (off the wsum critical path)
                ps_t2 = psum.tile([1, 128], F32, tag="psmall", bufs=2, name="ps_t2")
                nc.tensor.transpose(ps_t2[:], rsum[:], ident[:])
                gsum = spool.tile([1, 1], F32, tag="gsum")
                nc.vector.tensor_reduce(
                    out=gsum[:], in_=ps_t2[:], axis=mybir.AxisListType.X,
                    op=mybir.AluOpType.add,
                )
                rrow = spool.tile([1, 1], F32, tag="rrow")
                nc.vector.reciprocal(rrow[:], gsum[:])
                ps_b2 = psum.tile([128, 1], F32, tag="psmall", bufs=2, name="ps_b2")
                nc.tensor.matmul(ps_b2[:], lhsT=ones_row[:], rhs=rrow[:],
                                 start=True, stop=True)
                rtot = spool.tile([128, 1], F32, tag="rtot")
                nc.scalar.copy(rtot[:], ps_b2[:])

                # normalized weights for the attentions output
                w_sb = spool.tile([128, K], F32, tag="wsb")
                nc.vector.tensor_scalar_mul(w_sb[:], e_sb[:], rtot[:])

                ctx_sb = spool.tile([1, D], F32, tag="ctxsb")
                nc.vector.scalar_tensor_tensor(
                    out=ctx_sb[:],
                    in0=ps_ctx[:],
                    scalar=rrow[:],
                    in1=rproj[b][:],
                    op0=mybir.AluOpType.mult,
                    op1=mybir.AluOpType.mult,
                )

                # attn row layout: transpose w [128,K] -> [K,128]
                ps_wt = psum.tile([K, 128], F32, tag="pswt", bufs=1)
                nc.tensor.transpose(ps_wt[:], w_sb[:], ident[:])
                wrow = spool.tile([K, 128], F32, tag="wrow")
                nc.scalar.copy(wrow[:], ps_wt[:])

                # bounce rows to DRAM, then broadcast T-fold with step-0 source
                nc.sync.dma_start(out=dctx[b, :], in_=ctx_sb[:])
                nc.sync.dma_start(
                    out=dattn.rearrange("b (k p) -> b k p", p=128)[b], in_=wrow[:]
                )
                cbase = dctx[b, :]
                bctx = bass.AP(
                    tensor=cbase.tensor, offset=cbase.offset,
                    ap=[[0, T]] + list(cbase.ap),
                )
                nc.sync.dma_start(out=octx[:, b, :], in_=bctx)
                abase = dattn[b, :]
                battn = bass.AP(
                    tensor=abase.tensor, offset=abase.offset,
                    ap=[[0, T]] + list(abase.ap),
                )
                nc.scalar.dma_start(out=oattn[:, b, :], in_=battn)

            ppool_cm.__exit__(None, None, None)

    _split_excess_waits(nc)
    return nc


_NC_CACHE = {}


def _get_nc(T):
    if T not in _NC_CACHE:
        _NC_CACHE[T] = build_nc(T=T)
    return _NC_CACHE[T]


def kernel(seqlen, hidden, contextvects, W, padding_mask):
    T = int(np.asarray(seqlen))
    hidden = np.asarray(hidden, dtype=np.float32)
    contextvects = np.ascontiguousarray(np.asarray(contextvects, dtype=np.float32))
    W = np.ascontiguousarray(np.asarray(W, dtype=np.float32))
    mask_u8 = np.ascontiguousarray(
        np.asarray(padding_mask).astype(np.uint8, copy=False)
    )

    Bfull = hidden.shape[1]
    assert Bfull % N_CORES == 0
    Bc = Bfull // N_CORES

    nc = _get_nc(T)
    in_maps = []
    for i in range(N_CORES):
        sl = slice(i * Bc, (i + 1) * Bc)
        in_maps.append(
            {
                "hidden": np.ascontiguousarray(hidden[:, sl, :]),
                "contextvects": np.ascontiguousarray(contextvects[:, sl, :]),
                "W": W,
                "padding_mask": mask_u8[:, sl],
            }
        )
    res = run_bass_kernel_spmd(nc, in_maps, core_ids=list(range(N_CORES)))
    context = np.concatenate([r["context"] for r in res.results], axis=1)
    attn = np.concatenate([r["attn"] for r in res.results], axis=1)
    return context, attn


if __name__ == "__main__":
    # quick smoke test with random data (no reference available here)
    rng = np.random.default_rng(0)
    inputs = {
        "seqlen": np.int64(64),
        "hidden": rng.standard_normal((1, 32, 1024), dtype=np.float32),
        "contextvects": rng.standard_normal((2048, 32, 1024), dtype=np.float32),
        "W": (rng.standard_normal((1024, 1024), dtype=np.float32) / 32.0),
        "padding_mask": np.zeros((2048, 32), dtype=bool),
    }
    ctx, attn = kernel(**inputs)
    print("context", ctx.shape, ctx.dtype, "attn", attn.shape, attn.dtype)


# revision 19
# speedup vs baseline: 50953.8413x; 1.0238x over previous
"""Trainium2 Bass kernel for nn_AttentionMechanism (sparse_attention).

Reference computation (per full batch B=32):
    h          = hidden[0]                         # [B, H]
    proj       = h @ W.T                           # [B, D]
    scores     = einsum('cbd,bd->cb', ctx, proj)   # [C, B]
    scores     = where(mask, -inf, scores)
    attn       = softmax(scores, axis=0)           # [C, B]
    ctxout     = einsum('cb,cbd->bd', attn, ctx)   # [B, D]
    context    = broadcast ctxout  -> [T, B, D]
    attentions = broadcast attn.T  -> [T, B, C]

Sharding: data-parallel over batch across 8 NeuronCores (4 batches/core),
W replicated.  Per-core kernel keeps contextvects in natural layout
[c(part), d(free)]:
  - scores: fused multiply+reduce on DVE (contract d along free dim)
  - softmax: DVE row-max -> GPSIMD partition max -> ACT fused exp+rowsum
             -> PE ones-matmul partition sum is replaced by GPSIMD add
  - weighted sum: PE matmuls (contract c = partition dim), M=1, PSUM acc
  - seqlen broadcast done by step-0 source DMA from a DRAM bounce row
"""

import os
import sys

for _p in ("/opt/trn_rl_repo", "/root/.axon_site/_ro/trn_rl_repo"):
    if os.path.isdir(_p) and _p not in sys.path:
        sys.path.insert(0, _p)

import numpy as np

import concourse.bass as bass
import concourse.tile as tile
from concourse import mybir
import concourse.bass_isa as bass_isa
from concourse.bass_utils import run_bass_kernel_spmd
from concourse.masks import make_identity

F32 = mybir.dt.float32
F32R = mybir.dt.float32r
U8 = mybir.dt.uint8

N_CORES = 8


_MAX_WAITS = 1


def _split_excess_waits(nc):
    """This container's walrus rejects instructions carrying more than ~2 sem
    waits.  Move excess waits onto same-engine NOPs inserted just before the
    offending instruction (waits still happen-before it in program order)."""
    n_split = 0
    for f in nc.m.functions:
        for bb in f.blocks:
            new_insts = []
            for ins in bb.instructions:
                si = ins.sync_info
                if si is not None and si.on_wait and len(si.on_wait) > _MAX_WAITS:
                    waits = list(si.on_wait)
                    keep = waits[: _MAX_WAITS]
                    rest = waits[_MAX_WAITS:]
                    si.on_wait.clear()
                    for w in keep:
                        si.on_wait.append(w)
                    for j, w in enumerate(rest):
                        nop = mybir.InstNoOp(
                            name=f"{ins.name}-wsplit{j}", ins=[], outs=[]
                        )
                        nop.engine = ins.engine
                        nop.sync_info = mybir.SyncInfo(on_wait=[w], on_update=[])
                        new_insts.append(nop)
                        n_split += 1
                new_insts.append(ins)
            bb.instructions.clear()
            for i in new_insts:
                bb.instructions.append(i)
    return n_split


def build_nc(T=64, C=2048, D=1024, H=1024, B=4, trace_sim=False):
    """Build the per-core bass program (B = batches on this core)."""
    assert C % 128 == 0 and D % 512 == 0 and H % 1024 == 0
    K = C // 128          # context tiles (partition dim c)
    HT = H // 128         # h contraction tiles
    DT = D // 128         # dout tiles of W (natural layout)
    NCH = D // 512        # 512-wide output chunks

    nc = bass.Bass()
    hid = nc.declare_dram_parameter("hidden", [1, B, H], F32, isOutput=False)
    cvec = nc.declare_dram_parameter("contextvects", [C, B, D], F32, isOutput=False)
    Wp = nc.declare_dram_parameter("W", [D, H], F32, isOutput=False)
    maskp = nc.declare_dram_parameter("padding_mask", [C, B], U8, isOutput=False)
    octx = nc.declare_dram_parameter("context", [T, B, D], F32, isOutput=True)
    oattn = nc.declare_dram_parameter("attn", [T, B, C], F32, isOutput=True)

    # DRAM bounce rows for the T-fold broadcast writes
    dctx = nc.dram_tensor("bounce_ctx", [B, D], F32)
    dattn = nc.dram_tensor("bounce_attn", [B, C], F32)

    with tile.TileContext(nc, trace_sim=trace_sim) as tc:
        with (
            tc.tile_pool(name="singles", bufs=1) as singles,
            tc.tile_pool(name="cpool", bufs=8) as cpool,
            tc.tile_pool(name="spool", bufs=2) as spool,
            tc.tile_pool(name="psum", bufs=1, space="PSUM") as psum,
        ):
            # ---------- phase 0: constants, W^T, h, proj ----------
            ident = singles.tile([128, 128], F32)
            make_identity(nc, ident)
            ones_row = singles.tile([1, 128], F32)
            nc.vector.memset(ones_row, 1.0)
            neg_tile = singles.tile([128, K], F32)
            nc.vector.memset(neg_tile, -1e30)

            # mask, whole shard in one DMA: [p, k, b]
            mask_sb = singles.tile([128, K, B], U8)
            nc.sync.dma_start(
                out=mask_sb[:], in_=maskp.rearrange("(k p) b -> p k b", p=128)
            )

            # h in interleaved layout: hall[p, b, f] holds h_b[p*HT + f]
            hall = singles.tile([128, B, HT], F32)
            for b in range(B):
                nc.sync.dma_start(
                    out=hall[:, b, :],
                    in_=hid[0:1, b, :],
                )

            # W^T tiles: wt[f][r, dout] = W[dout, HT*r + f]
            wtp_cm = tc.tile_pool(name="wtp", bufs=1)
            wtp = wtp_cm.__enter__()
            wnatp_cm = tc.tile_pool(name="wnatp", bufs=8)
            wnatp = wnatp_cm.__enter__()
            wt = []
            for f in range(HT):
                wt.append(wtp.tile([128, D], F32, tag=f"wt{f}", name=f"wt{f}"))
            wns = []
            for dt_ in range(DT):
                wn = wnatp.tile([128, H], F32, tag="wnat", name=f"wn{dt_}")
                eng = nc.sync if dt_ % 2 == 0 else nc.scalar
                eng.dma_start(out=wn[:], in_=Wp[dt_ * 128 : (dt_ + 1) * 128, :])
                wns.append(wn)

            # --- fast path for batch 0's proj on DVE (PE is busy transposing W;
            # this lets batch-0 scores start ~35us earlier) ---
            h0row = spool.tile([1, H], F32, tag="h0row")
            nc.sync.dma_start(out=h0row[:], in_=hid[0:1, 0, :])
            ps_h0 = psum.tile([128, H], F32, tag="pswide", bufs=2, name="ps_h0")
            for ch in range(H // 512):
                nc.tensor.matmul(
                    ps_h0[:, ch * 512 : (ch + 1) * 512],
                    lhsT=ones_row[:],
                    rhs=h0row[:, ch * 512 : (ch + 1) * 512],
                    start=True, stop=True,
                )
            h0b = singles.tile([128, H], F32)
            nc.scalar.copy(h0b[:], ps_h0[:])
            projT0 = singles.tile([128, DT], F32)
            junk0 = singles.tile([128, H], F32)
            for dt_ in range(DT):
                nc.vector.scalar_tensor_tensor(
                    out=junk0[:],
                    in0=wns[dt_][:],
                    scalar=0.0,
                    in1=h0b[:],
                    op0=mybir.AluOpType.bypass,
                    op1=mybir.AluOpType.mult,
                    accum_out=projT0[:, dt_ : dt_ + 1],
                )
            ps_pr0 = psum.tile([1, D], F32, tag="pswide", bufs=2, name="ps_pr0")
            for dt_ in range(DT):
                nc.tensor.transpose(
                    ps_pr0[0:1, dt_ * 128 : (dt_ + 1) * 128],
                    projT0[:, dt_ : dt_ + 1],
                    ident[:],
                )
            prow0 = spool.tile([1, D], F32, tag="projrow", name="prow0f")
            nc.scalar.copy(prow0[:], ps_pr0[:])
            rproj0 = singles.tile([1, D], F32, name="rproj0f")
            nc.vector.reciprocal(rproj0[:], prow0[:])
            ps_pb0 = psum.tile([128, D], F32, tag="pswide", bufs=2, name="ps_pb0")
            for ch in range(NCH):
                nc.tensor.matmul(
                    ps_pb0[:, ch * 512 : (ch + 1) * 512],
                    lhsT=ones_row[:],
                    rhs=prow0[:, ch * 512 : (ch + 1) * 512],
                    start=True, stop=True,
                )
            pb0 = singles.tile([128, D], F32, name="pb0f")
            nc.scalar.copy(pb0[:], ps_pb0[:])

            for dt_ in range(DT):
                wn = wns[dt_]
                for f in range(HT):
                    ps = psum.tile([128, 128], F32, tag="wtps", bufs=1)
                    # strided column view: h = f, f+HT, f+2*HT, ... (128 cols)
                    nc.tensor.transpose(ps[:], wn[:, f::HT], ident[:])
                    dst = wt[f][:, dt_ * 128 : (dt_ + 1) * 128]
                    if (dt_ + f) % 2 == 0:
                        nc.scalar.copy(out=dst, in_=ps[:])
                    else:
                        nc.vector.tensor_copy(out=dst, in_=ps[:])

            # proj for all B batches in one M=B chain: psum [B, D]
            ps_proj = psum.tile([B, D], F32, tag="pswide", bufs=2, name="ps_proj")
            for ch in range(NCH):
                for f in range(HT):
                    nc.tensor.matmul(
                        ps_proj[:, ch * 512 : (ch + 1) * 512],
                        lhsT=hall[:, :, f],
                        rhs=wt[f][:, ch * 512 : (ch + 1) * 512],
                        start=(f == 0),
                        stop=(f == HT - 1),
                    )
            proj_sb = singles.tile([B, D], F32)
            nc.scalar.copy(proj_sb[:], ps_proj[:])

            # move row b to partition 0 (SBUF->SBUF DMA), then ones-matmul
            # broadcast to all 128 partitions
            pb = [pb0]
            rproj = [rproj0]
            for b in range(1, B):
                prow = spool.tile([1, D], F32, tag="projrow", name=f"prow{b}")
                nc.sync.dma_start(out=prow[:], in_=proj_sb[b : b + 1, :])
                rproj.append(
                    singles.tile([1, D], F32, tag=f"rproj{b}", name=f"rproj{b}")
                )
                nc.vector.reciprocal(rproj[b][:], prow[:])
                ps_pb = psum.tile([128, D], F32, tag="pswide", bufs=2, name="ps_pb")
                for ch in range(NCH):
                    nc.tensor.matmul(
                        ps_pb[:, ch * 512 : (ch + 1) * 512],
                        lhsT=ones_row[:],
                        rhs=prow[:, ch * 512 : (ch + 1) * 512],
                        start=True, stop=True,
                    )
                pb.append(singles.tile([128, D], F32, tag=f"pb{b}", name=f"pb{b}"))
                nc.scalar.copy(pb[b][:], ps_pb[:])

            wnatp_cm.__exit__(None, None, None)
            wtp_cm.__exit__(None, None, None)
            ppool_cm = tc.tile_pool(name="ppool", bufs=18)
            ppool = ppool_cm.__enter__()

            # ---------- per-batch main loop ----------
            for b in range(B):
                # load C_b: K tiles [128, D] in natural layout
                cbk = []
                for k in range(K):
                    cb = cpool.tile([128, D], F32, tag="cb")
                    eng = nc.sync if k % 2 == 0 else nc.scalar
                    eng.dma_start(
                        out=cb[:], in_=cvec[k * 128 : (k + 1) * 128, b, :]
                    )
                    cbk.append(cb)

                # scores: fused multiply + free-dim reduce on DVE.  The
                # elementwise product P[c,d] = C[c,d]*proj[d] is kept (in
                # f32r) as the weighted-sum input; ctx divides by proj later.
                scores = spool.tile([128, K], F32, tag="scores")
                pk = []
                for k in range(K):
                    pt = ppool.tile([128, D], F32R, tag="pk", name=f"pk{k}")
                    nc.vector.scalar_tensor_tensor(
                        out=pt[:],
                        in0=cbk[k][:],
                        scalar=0.0,
                        in1=pb[b][:],
                        op0=mybir.AluOpType.bypass,
                        op1=mybir.AluOpType.mult,
                        accum_out=scores[:, k : k + 1],
                    )
                    pk.append(pt)

                # padding mask -> -1e30
                nc.vector.copy_predicated(
                    out=scores[:], mask=mask_sb[:, :, b], data=neg_tile[:]
                )

                # softmax over all C entries (partitions x K)
                rmax = spool.tile([128, 1], F32, tag="rmax")
                nc.vector.tensor_reduce(
                    out=rmax[:], in_=scores[:], axis=mybir.AxisListType.X,
                    op=mybir.AluOpType.max,
                )
                ps_t1 = psum.tile([1, 128], F32, tag="psmall", bufs=2, name="ps_t1")
                nc.tensor.transpose(ps_t1[:], rmax[:], ident[:])
                gmax = spool.tile([1, 1], F32, tag="gmax")
                nc.vector.tensor_reduce(
                    out=gmax[:], in_=ps_t1[:], axis=mybir.AxisListType.X,
                    op=mybir.AluOpType.max,
                )
                ps_b1 = psum.tile([128, 1], F32, tag="psmall", bufs=2, name="ps_b1")
                nc.tensor.matmul(ps_b1[:], lhsT=ones_row[:], rhs=gmax[:],
                                 start=True, stop=True)
                nmax = spool.tile([128, 1], F32, tag="nmax")
                nc.scalar.mul(nmax[:], ps_b1[:], -1.0)

                e_sb = spool.tile([128, K], F32, tag="esb")
                rsum = spool.tile([128, 1], F32, tag="rsum")
                nc.scalar.activation(
                    out=e_sb[:], in_=scores[:],
                    func=mybir.ActivationFunctionType.Exp,
                    bias=nmax[:], scale=1.0, accum_out=rsum[:],
                )
                # weighted sum on PE in f32r (full-rate streaming), using the
                # P product tiles; UNNORMALIZED e as stationary weights.
                e_r = spool.tile([128, K], F32R, tag="er")
                nc.scalar.copy(e_r[:], e_sb[:])
                ps_ctx = psum.tile([1, D], F32, tag="pswide", bufs=2, name="ps_ctx")
                for ch in range(NCH):
                    for k in range(K):
                        nc.tensor.matmul(
                            ps_ctx[:, ch * 512 : (ch + 1) * 512],
                            lhsT=e_r[:, k : k + 1],
                            rhs=pk[k][:, ch * 512 : (ch + 1) * 512],
                            start=(k == 0),
                            stop=(k == K - 1),
                        )

                # total sum + reciprocal (off the wsum critical path)
                ps_t2 = psum.tile([1, 128], F32, tag="psmall", bufs=2, name="ps_t2")
                nc.tensor.transpose(ps_t2[:], rsum[:], ident[:])
                gsum = spool.tile([1, 1], F32, tag="gsum")
                nc.vector.tensor_reduce(
                    out=gsum[:], in_=ps_t2[:], axis=mybir.AxisListType.X,
                    op=mybir.AluOpType.add,
                )
                rrow = spool.tile([1, 1], F32, tag="rrow")
                nc.vector.reciprocal(rrow[:], gsum[:])
                ps_b2 = psum.tile([128, 1], F32, tag="psmall", bufs=2, name="ps_b2")
                nc.tensor.matmul(ps_b2[:], lhsT=ones_row[:], rhs=rrow[:],
                                 start=True, stop=True)
                rtot = spool.tile([128, 1], F32, tag="rtot")
                nc.scalar.copy(rtot[:], ps_b2[:])

                # normalized weights for the attentions output
                w_sb = spool.tile([128, K], F32, tag="wsb")
                nc.vector.tensor_scalar_mul(w_sb[:], e_sb[:], rtot[:])

                ctx_sb = spool.tile([1, D], F32, tag="ctxsb")
                nc.vector.scalar_tensor_tensor(
                    out=ctx_sb[:],
                    in0=ps_ctx[:],
                    scalar=rrow[:],
                    in1=rproj[b][:],
                    op0=mybir.AluOpType.mult,
                    op1=mybir.AluOpType.mult,
                )

                # attn row layout: transpose w [128,K] -> [K,128]
                ps_wt = psum.tile([K, 128], F32, tag="pswt", bufs=1)
                nc.tensor.transpose(ps_wt[:], w_sb[:], ident[:])
                wrow = spool.tile([K, 128], F32, tag="wrow")
                nc.scalar.copy(wrow[:], ps_wt[:])

                # bounce rows to DRAM, then broadcast T-fold with step-0 source
                nc.sync.dma_start(out=dctx[b, :], in_=ctx_sb[:])
                nc.sync.dma_start(
                    out=dattn.rearrange("b (k p) -> b k p", p=128)[b], in_=wrow[:]
                )
                cbase = dctx[b, :]
                bctx = bass.AP(
                    tensor=cbase.tensor, offset=cbase.offset,
                    ap=[[0, T]] + list(cbase.ap),
                )
                nc.sync.dma_start(out=octx[:, b, :], in_=bctx)
                abase = dattn[b, :]
                battn = bass.AP(
                    tensor=abase.tensor, offset=abase.offset,
                    ap=[[0, T]] + list(abase.ap),
                )
                nc.scalar.dma_start(out=oattn[:, b, :], in_=battn)

            ppool_cm.__exit__(None, None, None)

    _split_excess_waits(nc)
    return nc


_NC_CACHE = {}


def _get_nc(T):
    if T not in _NC_CACHE:
        _NC_CACHE[T] = build_nc(T=T)
    return _NC_CACHE[T]


def kernel(seqlen, hidden, contextvects, W, padding_mask):
    T = int(np.asarray(seqlen))
    hidden = np.asarray(hidden, dtype=np.float32)
    contextvects = np.ascontiguousarray(np.asarray(contextvects, dtype=np.float32))
    W = np.ascontiguousarray(np.asarray(W, dtype=np.float32))
    mask_u8 = np.ascontiguousarray(
        np.asarray(padding_mask).astype(np.uint8, copy=False)
    )

    Bfull = hidden.shape[1]
    assert Bfull % N_CORES == 0
    Bc = Bfull // N_CORES

    nc = _get_nc(T)
    in_maps = []
    for i in range(N_CORES):
        sl = slice(i * Bc, (i + 1) * Bc)
        in_maps.append(
            {
                "hidden": np.ascontiguousarray(hidden[:, sl, :]),
                "contextvects": np.ascontiguousarray(contextvects[:, sl, :]),
                "W": W,
                "padding_mask": mask_u8[:, sl],
            }
        )
    res = run_bass_kernel_spmd(nc, in_maps, core_ids=list(range(N_CORES)))
    context = np.concatenate([r["context"] for r in res.results], axis=1)
    attn = np.concatenate([r["attn"] for r in res.results], axis=1)
    return context, attn


if __name__ == "__main__":
    # quick smoke test with random data (no reference available here)
    rng = np.random.default_rng(0)
    inputs = {
        "seqlen": np.int64(64),
        "hidden": rng.standard_normal((1, 32, 1024), dtype=np.float32),
        "contextvects": rng.standard_normal((2048, 32, 1024), dtype=np.float32),
        "W": (rng.standard_normal((1024, 1024), dtype=np.float32) / 32.0),
        "padding_mask": np.zeros((2048, 32), dtype=bool),
    }
    ctx, attn = kernel(**inputs)
    print("context", ctx.shape, ctx.dtype, "attn", attn.shape, attn.dtype)


# revision 20
# speedup vs baseline: 56419.0330x; 1.1073x over previous
"""Trainium2 Bass kernel for nn_AttentionMechanism (sparse_attention).

Reference computation (per full batch B=32):
    h          = hidden[0]                         # [B, H]
    proj       = h @ W.T                           # [B, D]
    scores     = einsum('cbd,bd->cb', ctx, proj)   # [C, B]
    scores     = where(mask, -inf, scores)
    attn       = softmax(scores, axis=0)           # [C, B]
    ctxout     = einsum('cb,cbd->bd', attn, ctx)   # [B, D]
    context    = broadcast ctxout  -> [T, B, D]
    attentions = broadcast attn.T  -> [T, B, C]

Sharding: data-parallel over batch across 8 NeuronCores (4 batches/core),
W replicated.  Per-core kernel keeps contextvects in natural layout
[c(part), d(free)]:
  - scores: fused multiply+reduce on DVE (contract d along free dim)
  - softmax: DVE row-max -> GPSIMD partition max -> ACT fused exp+rowsum
             -> PE ones-matmul partition sum is replaced by GPSIMD add
  - weighted sum: PE matmuls (contract c = partition dim), M=1, PSUM acc
  - seqlen broadcast done by step-0 source DMA from a DRAM bounce row
"""

import os
import sys

for _p in ("/opt/trn_rl_repo", "/root/.axon_site/_ro/trn_rl_repo"):
    if os.path.isdir(_p) and _p not in sys.path:
        sys.path.insert(0, _p)

import numpy as np

import concourse.bass as bass
import concourse.tile as tile
from concourse import mybir
import concourse.bass_isa as bass_isa
from concourse.bass_utils import run_bass_kernel_spmd
from concourse.masks import make_identity

F32 = mybir.dt.float32
F32R = mybir.dt.float32r
U8 = mybir.dt.uint8

N_CORES = 8


_MAX_WAITS = 1


def _split_excess_waits(nc):
    """This container's walrus rejects instructions carrying more than ~2 sem
    waits.  Move excess waits onto same-engine NOPs inserted just before the
    offending instruction (waits still happen-before it in program order)."""
    n_split = 0
    for f in nc.m.functions:
        for bb in f.blocks:
            new_insts = []
            for ins in bb.instructions:
                si = ins.sync_info
                if si is not None and si.on_wait and len(si.on_wait) > _MAX_WAITS:
                    waits = list(si.on_wait)
                    keep = waits[: _MAX_WAITS]
                    rest = waits[_MAX_WAITS:]
                    si.on_wait.clear()
                    for w in keep:
                        si.on_wait.append(w)
                    for j, w in enumerate(rest):
                        nop = mybir.InstNoOp(
                            name=f"{ins.name}-wsplit{j}", ins=[], outs=[]
                        )
                        nop.engine = ins.engine
                        nop.sync_info = mybir.SyncInfo(on_wait=[w], on_update=[])
                        new_insts.append(nop)
                        n_split += 1
                new_insts.append(ins)
            bb.instructions.clear()
            for i in new_insts:
                bb.instructions.append(i)
    return n_split


def build_nc(T=64, C=2048, D=1024, H=1024, B=4, trace_sim=False):
    """Build the per-core bass program (B = batches on this core)."""
    assert C % 128 == 0 and D % 512 == 0 and H % 1024 == 0
    K = C // 128          # context tiles (partition dim c)
    HT = H // 128         # h contraction tiles
    DT = D // 128         # dout tiles of W (natural layout)
    NCH = D // 512        # 512-wide output chunks

    nc = bass.Bass()
    hid = nc.declare_dram_parameter("hidden", [1, B, H], F32, isOutput=False)
    cvec = nc.declare_dram_parameter("contextvects", [C, B, D], F32, isOutput=False)
    Wp = nc.declare_dram_parameter("W", [D, H], F32, isOutput=False)
    maskp = nc.declare_dram_parameter("padding_mask", [C, B], U8, isOutput=False)
    octx = nc.declare_dram_parameter("context", [T, B, D], F32, isOutput=True)
    oattn = nc.declare_dram_parameter("attn", [T, B, C], F32, isOutput=True)

    # DRAM bounce rows for the T-fold broadcast writes
    dctx = nc.dram_tensor("bounce_ctx", [B, D], F32)
    dattn = nc.dram_tensor("bounce_attn", [B, C], F32)

    with tile.TileContext(nc, trace_sim=trace_sim) as tc:
        with (
            tc.tile_pool(name="singles", bufs=1) as singles,
            tc.tile_pool(name="cpool", bufs=10) as cpool,
            tc.tile_pool(name="spool", bufs=2) as spool,
            tc.tile_pool(name="psum", bufs=1, space="PSUM") as psum,
        ):
            # ---------- phase 0: constants, W^T, h, proj ----------
            ident = singles.tile([128, 128], F32)
            make_identity(nc, ident)
            ones_row = singles.tile([1, 128], F32)
            nc.vector.memset(ones_row, 1.0)
            neg_tile = singles.tile([128, K], F32)
            nc.vector.memset(neg_tile, -1e30)

            # mask, whole shard in one DMA: [p, k, b]
            mask_sb = singles.tile([128, K, B], U8)
            nc.sync.dma_start(
                out=mask_sb[:], in_=maskp.rearrange("(k p) b -> p k b", p=128)
            )

            # h in interleaved layout: hall[p, b, f] holds h_b[p*HT + f]
            hall = singles.tile([128, B, HT], F32)
            for b in range(B):
                nc.sync.dma_start(
                    out=hall[:, b, :],
                    in_=hid[0:1, b, :],
                )

            # W^T tiles: wt[f][r, dout] = W[dout, HT*r + f]
            wtp_cm = tc.tile_pool(name="wtp", bufs=1)
            wtp = wtp_cm.__enter__()
            wnatp_cm = tc.tile_pool(name="wnatp", bufs=8)
            wnatp = wnatp_cm.__enter__()
            wt = []
            for f in range(HT):
                wt.append(wtp.tile([128, D], F32, tag=f"wt{f}", name=f"wt{f}"))
            wns = []
            for dt_ in range(DT):
                wn = wnatp.tile([128, H], F32, tag="wnat", name=f"wn{dt_}")
                eng = nc.sync if dt_ % 2 == 0 else nc.scalar
                eng.dma_start(out=wn[:], in_=Wp[dt_ * 128 : (dt_ + 1) * 128, :])
                wns.append(wn)

            # --- fast path for batch 0's proj on DVE (PE is busy transposing W;
            # this lets batch-0 scores start ~35us earlier) ---
            h0row = spool.tile([1, H], F32, tag="h0row")
            nc.sync.dma_start(out=h0row[:], in_=hid[0:1, 0, :])
            ps_h0 = psum.tile([128, H], F32, tag="pswide", bufs=2, name="ps_h0")
            for ch in range(H // 512):
                nc.tensor.matmul(
                    ps_h0[:, ch * 512 : (ch + 1) * 512],
                    lhsT=ones_row[:],
                    rhs=h0row[:, ch * 512 : (ch + 1) * 512],
                    start=True, stop=True,
                )
            h0b = singles.tile([128, H], F32)
            nc.scalar.copy(h0b[:], ps_h0[:])
            projT0 = singles.tile([128, DT], F32)
            junk0 = singles.tile([128, H], F32)
            for dt_ in range(DT):
                nc.vector.scalar_tensor_tensor(
                    out=junk0[:],
                    in0=wns[dt_][:],
                    scalar=0.0,
                    in1=h0b[:],
                    op0=mybir.AluOpType.bypass,
                    op1=mybir.AluOpType.mult,
                    accum_out=projT0[:, dt_ : dt_ + 1],
                )
            ps_pr0 = psum.tile([1, D], F32, tag="pswide", bufs=2, name="ps_pr0")
            for dt_ in range(DT):
                nc.tensor.transpose(
                    ps_pr0[0:1, dt_ * 128 : (dt_ + 1) * 128],
                    projT0[:, dt_ : dt_ + 1],
                    ident[:],
                )
            prow0 = spool.tile([1, D], F32, tag="projrow", name="prow0f")
            nc.scalar.copy(prow0[:], ps_pr0[:])
            rproj0 = singles.tile([1, D], F32, name="rproj0f")
            nc.vector.reciprocal(rproj0[:], prow0[:])
            ps_pb0 = psum.tile([128, D], F32, tag="pswide", bufs=2, name="ps_pb0")
            for ch in range(NCH):
                nc.tensor.matmul(
                    ps_pb0[:, ch * 512 : (ch + 1) * 512],
                    lhsT=ones_row[:],
                    rhs=prow0[:, ch * 512 : (ch + 1) * 512],
                    start=True, stop=True,
                )
            pb0 = singles.tile([128, D], F32, name="pb0f")
            nc.scalar.copy(pb0[:], ps_pb0[:])

            for f in range(HT):
                for g in range(DT // 4):
                    ps = psum.tile([128, 512], F32, tag="wtps", bufs=2)
                    for j in range(4):
                        # strided column view: h = f, f+HT, f+2*HT, ...
                        nc.tensor.transpose(
                            ps[:, j * 128 : (j + 1) * 128],
                            wns[4 * g + j][:, f::HT],
                            ident[:],
                        )
                    nc.scalar.copy(
                        out=wt[f][:, g * 512 : (g + 1) * 512], in_=ps[:]
                    )

            # proj for all B batches in one M=B chain: psum [B, D]
            ps_proj = psum.tile([B, D], F32, tag="pswide", bufs=2, name="ps_proj")
            for ch in range(NCH):
                for f in range(HT):
                    nc.tensor.matmul(
                        ps_proj[:, ch * 512 : (ch + 1) * 512],
                        lhsT=hall[:, :, f],
                        rhs=wt[f][:, ch * 512 : (ch + 1) * 512],
                        start=(f == 0),
                        stop=(f == HT - 1),
                    )
            proj_sb = singles.tile([B, D], F32)
            nc.scalar.copy(proj_sb[:], ps_proj[:])

            # move row b to partition 0 (SBUF->SBUF DMA), then ones-matmul
            # broadcast to all 128 partitions
            pb = [pb0]
            rproj = [rproj0]
            for b in range(1, B):
                prow = spool.tile([1, D], F32, tag="projrow", name=f"prow{b}")
                nc.sync.dma_start(out=prow[:], in_=proj_sb[b : b + 1, :])
                rproj.append(
                    singles.tile([1, D], F32, tag=f"rproj{b}", name=f"rproj{b}")
                )
                nc.vector.reciprocal(rproj[b][:], prow[:])
                ps_pb = psum.tile([128, D], F32, tag="pswide", bufs=2, name="ps_pb")
                for ch in range(NCH):
                    nc.tensor.matmul(
                        ps_pb[:, ch * 512 : (ch + 1) * 512],
                        lhsT=ones_row[:],
                        rhs=prow[:, ch * 512 : (ch + 1) * 512],
                        start=True, stop=True,
                    )
                pb.append(singles.tile([128, D], F32, tag=f"pb{b}", name=f"pb{b}"))
                nc.scalar.copy(pb[b][:], ps_pb[:])

            wnatp_cm.__exit__(None, None, None)
            wtp_cm.__exit__(None, None, None)
            ppool_cm = tc.tile_pool(name="ppool", bufs=18)
            ppool = ppool_cm.__enter__()

            # ---------- per-batch main loop ----------
            for b in range(B):
                # load C_b: K tiles [128, D] in natural layout
                cbk = []
                for k in range(K):
                    cb = cpool.tile([128, D], F32, tag="cb")
                    eng = nc.sync if k % 2 == 0 else nc.scalar
                    eng.dma_start(
                        out=cb[:], in_=cvec[k * 128 : (k + 1) * 128, b, :]
                    )
                    cbk.append(cb)

                # scores: fused multiply + free-dim reduce on DVE.  The
                # elementwise product P[c,d] = C[c,d]*proj[d] is kept (in
                # f32r) as the weighted-sum input; ctx divides by proj later.
                scores = spool.tile([128, K], F32, tag="scores")
                pk = []
                for k in range(K):
                    pt = ppool.tile([128, D], F32R, tag="pk", name=f"pk{k}")
                    nc.vector.scalar_tensor_tensor(
                        out=pt[:],
                        in0=cbk[k][:],
                        scalar=0.0,
                        in1=pb[b][:],
                        op0=mybir.AluOpType.bypass,
                        op1=mybir.AluOpType.mult,
                        accum_out=scores[:, k : k + 1],
                    )
                    pk.append(pt)

                # padding mask -> -1e30
                nc.vector.copy_predicated(
                    out=scores[:], mask=mask_sb[:, :, b], data=neg_tile[:]
                )

                # softmax over all C entries (partitions x K)
                rmax = spool.tile([128, 1], F32, tag="rmax")
                nc.vector.tensor_reduce(
                    out=rmax[:], in_=scores[:], axis=mybir.AxisListType.X,
                    op=mybir.AluOpType.max,
                )
                ps_t1 = psum.tile([1, 128], F32, tag="psmall", bufs=1, name="ps_t1")
                nc.tensor.transpose(ps_t1[:], rmax[:], ident[:])
                gmax = spool.tile([1, 1], F32, tag="gmax")
                nc.vector.tensor_reduce(
                    out=gmax[:], in_=ps_t1[:], axis=mybir.AxisListType.X,
                    op=mybir.AluOpType.max,
                )
                ps_b1 = psum.tile([128, 1], F32, tag="psmall", bufs=1, name="ps_b1")
                nc.tensor.matmul(ps_b1[:], lhsT=ones_row[:], rhs=gmax[:],
                                 start=True, stop=True)
                nmax = spool.tile([128, 1], F32, tag="nmax")
                nc.scalar.mul(nmax[:], ps_b1[:], -1.0)

                e_sb = spool.tile([128, K], F32, tag="esb")
                rsum = spool.tile([128, 1], F32, tag="rsum")
                nc.scalar.activation(
                    out=e_sb[:], in_=scores[:],
                    func=mybir.ActivationFunctionType.Exp,
                    bias=nmax[:], scale=1.0, accum_out=rsum[:],
                )
                # weighted sum on PE in f32r (full-rate streaming), using the
                # P product tiles; UNNORMALIZED e as stationary weights.
                e_r = spool.tile([128, K], F32R, tag="er")
                nc.scalar.copy(e_r[:], e_sb[:])
                ps_ctx = psum.tile([1, D], F32, tag="pswide", bufs=2, name="ps_ctx")
                for ch in range(NCH):
                    for k in range(K):
                        nc.tensor.matmul(
                            ps_ctx[:, ch * 512 : (ch + 1) * 512],
                            lhsT=e_r[:, k : k + 1],
                            rhs=pk[k][:, ch * 512 : (ch + 1) * 512],
                            start=(k == 0),
                            stop=(k == K - 1),
                        )

                # total sum + reciprocal (off the wsum critical path)
                ps_t2 = psum.tile([1, 128], F32, tag="psmall", bufs=1, name="ps_t2")
                nc.tensor.transpose(ps_t2[:], rsum[:], ident[:])
                gsum = spool.tile([1, 1], F32, tag="gsum")
                nc.vector.tensor_reduce(
                    out=gsum[:], in_=ps_t2[:], axis=mybir.AxisListType.X,
                    op=mybir.AluOpType.add,
                )
                rrow = spool.tile([1, 1], F32, tag="rrow")
                nc.vector.reciprocal(rrow[:], gsum[:])
                ps_b2 = psum.tile([128, 1], F32, tag="psmall", bufs=1, name="ps_b2")
                nc.tensor.matmul(ps_b2[:], lhsT=ones_row[:], rhs=rrow[:],
                                 start=True, stop=True)
                rtot = spool.tile([128, 1], F32, tag="rtot")
                nc.scalar.copy(rtot[:], ps_b2[:])

                # normalized weights for the attentions output
                w_sb = spool.tile([128, K], F32, tag="wsb")
                nc.vector.tensor_scalar_mul(w_sb[:], e_sb[:], rtot[:])

                ctx_sb = spool.tile([1, D], F32, tag="ctxsb")
                nc.vector.scalar_tensor_tensor(
                    out=ctx_sb[:],
                    in0=ps_ctx[:],
                    scalar=rrow[:],
                    in1=rproj[b][:],
                    op0=mybir.AluOpType.mult,
                    op1=mybir.AluOpType.mult,
                )

                # attn row layout: transpose w [128,K] -> [K,128]
                ps_wt = psum.tile([K, 128], F32, tag="pswt", bufs=1)
                nc.tensor.transpose(ps_wt[:], w_sb[:], ident[:])
                wrow = spool.tile([K, 128], F32, tag="wrow")
                nc.scalar.copy(wrow[:], ps_wt[:])

                # bounce rows to DRAM, then broadcast T-fold with step-0 source
                nc.sync.dma_start(out=dctx[b, :], in_=ctx_sb[:])
                nc.sync.dma_start(
                    out=dattn.rearrange("b (k p) -> b k p", p=128)[b], in_=wrow[:]
                )
                cbase = dctx[b, :]
                bctx = bass.AP(
                    tensor=cbase.tensor, offset=cbase.offset,
                    ap=[[0, T]] + list(cbase.ap),
                )
                nc.sync.dma_start(out=octx[:, b, :], in_=bctx)
                abase = dattn[b, :]
                battn = bass.AP(
                    tensor=abase.tensor, offset=abase.offset,
                    ap=[[0, T]] + list(abase.ap),
                )
                nc.scalar.dma_start(out=oattn[:, b, :], in_=battn)

            ppool_cm.__exit__(None, None, None)

    _split_excess_waits(nc)
    return nc


_NC_CACHE = {}


def _get_nc(T):
    if T not in _NC_CACHE:
        _NC_CACHE[T] = build_nc(T=T)
    return _NC_CACHE[T]


def kernel(seqlen, hidden, contextvects, W, padding_mask):
    T = int(np.asarray(seqlen))
    hidden = np.asarray(hidden, dtype=np.float32)
    contextvects = np.ascontiguousarray(np.asarray(contextvects, dtype=np.float32))
    W = np.ascontiguousarray(np.asarray(W, dtype=np.float32))
    mask_u8 = np.ascontiguousarray(
        np.asarray(padding_mask).astype(np.uint8, copy=False)
    )

    Bfull = hidden.shape[1]
    assert Bfull % N_CORES == 0
    Bc = Bfull // N_CORES

    nc = _get_nc(T)
    in_maps = []
    for i in range(N_CORES):
        sl = slice(i * Bc, (i + 1) * Bc)
        in_maps.append(
            {
                "hidden": np.ascontiguousarray(hidden[:, sl, :]),
                "contextvects": np.ascontiguousarray(contextvects[:, sl, :]),
                "W": W,
                "padding_mask": mask_u8[:, sl],
            }
        )
    res = run_bass_kernel_spmd(nc, in_maps, core_ids=list(range(N_CORES)))
    context = np.concatenate([r["context"] for r in res.results], axis=1)
    attn = np.concatenate([r["attn"] for r in res.results], axis=1)
    return context, attn


if __name__ == "__main__":
    # quick smoke test with random data (no reference available here)
    rng = np.random.default_rng(0)
    inputs = {
        "seqlen": np.int64(64),
        "hidden": rng.standard_normal((1, 32, 1024), dtype=np.float32),
        "contextvects": rng.standard_normal((2048, 32, 1024), dtype=np.float32),
        "W": (rng.standard_normal((1024, 1024), dtype=np.float32) / 32.0),
        "padding_mask": np.zeros((2048, 32), dtype=bool),
    }
    ctx, attn = kernel(**inputs)
    print("context", ctx.shape, ctx.dtype, "attn", attn.shape, attn.dtype)
